# revision 1
# baseline (speedup 1.0000x reference)
"""GAT layer (multi-head graph attention) on 8 TRN2 NeuronCores.

Strategy (per sharding hint): destination nodes are sharded across the 8
cores.  Each core:
  phase 1: computes the full projection table redundantly (bf16 GEMM
           X @ W.T plus the per-head attention score reductions), packed
           as [proj bf16 | s_src f32 | s_tgt f32 | pad] rows in local HBM.
  phase 2: walks its shard's destination windows (128 targets / window).
           Edges are pre-sorted by (window, src-bucket) on the host;
           dma_gather pulls the source rows (int16 indices per 32768-row
           bucket), scores -> leaky-relu -> exp run batched per window,
           and one-hot matmuls (host-streamed) accumulate both the
           softmax denominator and the weighted aggregation in PSUM.
           Softmax division + PReLU happen once per window at flush.

kernel(**inputs) takes the FULL inputs and returns the FULL output.
"""

import math
from dataclasses import dataclass, field

import numpy as np
import ml_dtypes

BF16 = ml_dtypes.bfloat16
P = 128


def _ceil(a, b):
    return -(-a // b)


@dataclass
class Cfg:
    N: int = 100000
    E: int = 800000
    HID: int = 512
    HEADS: int = 8
    ncores: int = 8
    bucket: int = 32768
    leak: float = 0.01
    oh_bf16: bool = True  # one-hot stream dtype (bf16; fp8 is an option)

    def __post_init__(self):
        assert self.N % self.ncores == 0
        assert self.bucket <= 32768
        self.F = self.HID // self.HEADS
        self.shard = self.N // self.ncores
        self.NW = _ceil(self.shard, P)          # windows per core
        self.NB = _ceil(self.N, self.bucket)    # src buckets (int16 range)
        self.NT = _ceil(self.N, P)              # projection tiles
        self.NPAD = self.NT * P
        self.KP = min(self.HID, P)              # contraction partitions
        self.KT = self.HID // self.KP           # contraction tiles
        row_bytes = self.HID * 2 + 2 * self.HEADS * 4
        self.row_bytes = _ceil(row_bytes, 256) * 256
        self.row_bf = self.row_bytes // 2
        self.row_f32 = self.row_bytes // 4
        self.s_src_off = self.HID // 2          # f32 col of s_src in a row
        self.s_tgt_off = self.HID // 2 + self.HEADS


@dataclass
class Schedule:
    """Core-independent (uniform) phase-2 schedule."""
    seg: np.ndarray          # [NW, NB] slot counts (128-aligned, global max)
    TW: list                 # tiles per window
    TWmax: int
    calls: list              # per window: list of (b, slot_off, nslots, idxcol0)
    idxcols: int             # total int16 idx columns (per 16-wrap row)
    TT: int                  # total tiles
    tile_base: list          # first global tile index of each window


def build_schedule(cfg: Cfg, counts: np.ndarray) -> Schedule:
    """counts: [ncores, NW, NB] edge counts."""
    maxcnt = counts.max(axis=0)  # [NW, NB]
    seg = np.where(maxcnt > 0, _ceil(maxcnt, P) * P, 0).astype(np.int64)
    TW, calls, tile_base = [], [], []
    idxcol = 0
    tt = 0
    for w in range(cfg.NW):
        tile_base.append(tt)
        wcalls = []
        off = 0
        for b in range(cfg.NB):
            s = int(seg[w, b])
            if s == 0:
                continue
            wcalls.append((b, off, s, idxcol))
            off += s
            idxcol += s // 16
        assert off % P == 0
        TW.append(off // P)
        tt += off // P
        calls.append(wcalls)
    return Schedule(seg=seg, TW=TW, TWmax=max(TW), calls=calls,
                    idxcols=idxcol, TT=tt, tile_base=tile_base)


def prep_core(cfg: Cfg, sched: Schedule, src, trg, k):
    """Per-core input arrays: g1 idx stream and one-hot stream."""
    oh_dt = BF16 if cfg.oh_bf16 else ml_dtypes.float8_e4m3
    mask = (trg // cfg.shard) == k
    esrc = src[mask]
    etrg = trg[mask]
    trel = etrg - k * cfg.shard
    win = trel // P
    buck = esrc // cfg.bucket
    # order edges by (window, bucket); stable so host/device agree
    order = np.lexsort((buck, win))
    esrc, etrg, trel, win, buck = (a[order] for a in (esrc, etrg, trel, win, buck))

    g1i = np.zeros((P, sched.idxcols), np.int16)
    oh = np.zeros((P, sched.TT, 2, P), oh_dt)

    # per (window, bucket) segment boundaries
    key = win * cfg.NB + buck
    # edge ranges per (w, b)
    starts = np.searchsorted(key, np.arange(cfg.NW * cfg.NB), side="left")
    ends = np.searchsorted(key, np.arange(cfg.NW * cfg.NB), side="right")

    for w in range(cfg.NW):
        for (b, slot_off, nslots, idxcol0) in sched.calls[w]:
            lo, hi = int(starts[w * cfg.NB + b]), int(ends[w * cfg.NB + b])
            cnt = hi - lo
            assert cnt <= nslots
            idx = np.zeros(nslots, np.int16)
            idx[:cnt] = (esrc[lo:hi] - b * cfg.bucket).astype(np.int16)
            blk = idx.reshape(nslots // 16, 16).T          # [16, cols]
            g1i[:, idxcol0:idxcol0 + nslots // 16] = np.tile(blk, (8, 1))
            # one-hots for this segment's tiles
            tloc = (trel[lo:hi] - w * P).astype(np.int64)  # [cnt] in [0,128)
            t0 = sched.tile_base[w] + slot_off // P
            for j in range(nslots // P):
                s0, s1 = j * P, min((j + 1) * P, cnt)
                if s1 <= s0:
                    continue
                rows = np.arange(s0, s1) - s0
                cols = tloc[s0:s1]
                oh[rows, t0 + j, 0, cols] = oh_dt(1.0)
                oh[cols, t0 + j, 1, rows] = oh_dt(1.0)
    return g1i, oh


def pack_xt(cfg: Cfg, X: np.ndarray) -> np.ndarray:
    """X [N, HID] f32 -> bf16 packed [KP, NT, KT, P]: (p, j, ki, n) = X[j*P+n, ki*KP+p]."""
    Xp = np.zeros((cfg.NPAD, cfg.HID), np.float32)
    Xp[: cfg.N] = X
    Xb = Xp.astype(BF16)
    # [NT, P(n), KT, KP(p)] -> transpose to [KP, NT, KT, P]
    v = Xb.reshape(cfg.NT, P, cfg.KT, cfg.KP)
    return np.ascontiguousarray(v.transpose(3, 0, 2, 1))


def pack_w(cfg: Cfg, W, a_src, a_tgt):
    """Returns wt [KP, KT, HID] bf16 and wa [KP, KT, 2*HEADS] bf16."""
    WT = W.T.astype(np.float32)                       # [HID(d), HID(o)]
    wa_s = (W.reshape(cfg.HEADS, cfg.F, cfg.HID)
            * np.asarray(a_src, np.float32).reshape(cfg.HEADS, cfg.F, 1)).sum(1)  # [H, d]
    wa_t = (W.reshape(cfg.HEADS, cfg.F, cfg.HID)
            * np.asarray(a_tgt, np.float32).reshape(cfg.HEADS, cfg.F, 1)).sum(1)
    WA = np.concatenate([wa_s.T, wa_t.T], axis=1)     # [d, 2H]
    wt = np.ascontiguousarray(
        WT.astype(BF16).reshape(cfg.KT, cfg.KP, cfg.HID).transpose(1, 0, 2))
    wa = np.ascontiguousarray(
        WA.astype(BF16).reshape(cfg.KT, cfg.KP, 2 * cfg.HEADS).transpose(1, 0, 2))
    return wt, wa


def _bcast_last(ap, n):
    """Append a 0-stride broadcast dim of size n to an AP."""
    import concourse.bass as bass
    lst = [list(x) for x in ap.ap] + [[0, n]]
    return bass.AP(ap.tensor, ap.offset, lst)


def build_nc(cfg: Cfg, sched: Schedule, phases: str = "full"):
    import concourse.bacc as bacc
    import concourse.bass as bass
    import concourse.mybir as mybir
    from concourse.tile import TileContext

    dt = mybir.dt
    oh_mdt = dt.bfloat16 if cfg.oh_bf16 else dt.float8e4
    H, HID, KT, KP = cfg.HEADS, cfg.HID, cfg.KT, cfg.KP

    nc = bacc.Bacc("TRN2", target_bir_lowering=False)

    xt = nc.dram_tensor("xt", [KP, cfg.NT, KT, P], dt.bfloat16, kind="ExternalInput")
    wt = nc.dram_tensor("wt", [KP, KT, HID], dt.bfloat16, kind="ExternalInput")
    wa = nc.dram_tensor("wa", [KP, KT, 2 * H], dt.bfloat16, kind="ExternalInput")
    g1i = nc.dram_tensor("g1i", [P, sched.idxcols], dt.int16, kind="ExternalInput")
    ohd = nc.dram_tensor("ohd", [P, sched.TT, 2, P], oh_mdt, kind="ExternalInput")
    avec = nc.dram_tensor("avec", [P, 1], dt.float32, kind="ExternalInput")
    out = nc.dram_tensor("out", [cfg.NW * P, HID], dt.float32, kind="ExternalOutput")

    with TileContext(nc) as tc:
        with tc.tile_pool(name="const", bufs=1) as cpool, \
             tc.tile_pool(name="dram", bufs=1, space="DRAM") as dpool:
            table = dpool.tile([cfg.NPAD, cfg.row_bf], dt.bfloat16)
            wt_sb = cpool.tile([KP, KT, HID], dt.bfloat16)
            nc.sync.dma_start(out=wt_sb[:], in_=wt[:, :, :])
            wa_sb = cpool.tile([KP, KT, 2 * H], dt.bfloat16)
            nc.sync.dma_start(out=wa_sb[:], in_=wa[:, :, :])
            if phases == "full":
                a_sb = cpool.tile([P, 1], dt.float32)
                nc.sync.dma_start(out=a_sb[:], in_=avec[:, :])
            if phases in ("full", "p1g"):
                g1i_sb = cpool.tile([P, sched.idxcols], dt.int16)
                nc.sync.dma_start(out=g1i_sb[:], in_=g1i[:, :])

            # ---------------- phase 1: projection table ----------------
            with tc.tile_pool(name="p1", bufs=3) as xpool, \
                 tc.tile_pool(name="p1ps", bufs=2, space="PSUM") as pspool, \
                 tc.tile_pool(name="p1st", bufs=3) as stpool:
                for j in range(cfg.NT):
                    xtile = xpool.tile([KP, KT, P], dt.bfloat16, tag="x")
                    nc.sync.dma_start(out=xtile[:], in_=xt[:, j, :, :])
                    ps1 = pspool.tile([P, HID], dt.float32, tag="ps1")
                    ps2 = pspool.tile([P, 2 * H], dt.float32, tag="ps2")
                    for ki in range(KT):
                        nc.tensor.matmul(ps1[:], xtile[:, ki, :], wt_sb[:, ki, :],
                                         start=(ki == 0), stop=(ki == KT - 1))
                    for ki in range(KT):
                        nc.tensor.matmul(ps2[:], xtile[:, ki, :], wa_sb[:, ki, :],
                                         start=(ki == 0), stop=(ki == KT - 1))
                    stg = stpool.tile([P, cfg.row_bf], dt.bfloat16, tag="stg")
                    stg32 = stg.bitcast(dt.float32)
                    nc.scalar.copy(out=stg[:, 0:HID], in_=ps1[:])
                    nc.scalar.copy(out=stg32[:, cfg.s_src_off:cfg.s_src_off + 2 * H],
                                   in_=ps2[:])
                    if cfg.s_tgt_off + H < cfg.row_f32:
                        nc.vector.memset(stg32[:, cfg.s_tgt_off + H:cfg.row_f32], 0.0)
                    nc.sync.dma_start(out=table[j * P:(j + 1) * P, :], in_=stg[:])

            tc.strict_bb_all_engine_barrier()

            if phases == "p1":
                with tc.tile_pool(name="dbg", bufs=2) as dbgp:
                    for w in range(cfg.NW):
                        res = dbgp.tile([P, HID], dt.float32, tag="res")
                        nc.vector.memset(res[:], 0.0)
                        nc.sync.dma_start(out=out[w * P:(w + 1) * P, :], in_=res[:])
                nc.compile()
                return nc

            if phases == "p1g":
                with tc.tile_pool(name="dbg", bufs=2) as dbgp:
                    for w in range(cfg.NW):
                        g1t = dbgp.tile([P, sched.TWmax, cfg.row_bf], dt.bfloat16,
                                        tag="g1t")
                        for (b, slot_off, nslots, idxcol0) in sched.calls[w]:
                            rows = min(cfg.NPAD, (b + 1) * cfg.bucket) - b * cfg.bucket
                            nc.gpsimd.dma_gather(
                                g1t[:, slot_off // P:(slot_off + nslots) // P, :],
                                table[b * cfg.bucket:b * cfg.bucket + rows, :],
                                g1i_sb[:, idxcol0:idxcol0 + nslots // 16],
                                nslots, nslots, cfg.row_bf)
                        res = dbgp.tile([P, HID], dt.float32, tag="res")
                        nc.vector.memset(res[:], 0.0)
                        nc.sync.dma_start(out=out[w * P:(w + 1) * P, :], in_=res[:])
                nc.compile()
                return nc

            # ---------------- phase 1.5: resident s_tgt (hi/lo bf16) ----------------
            pid = nc.sync.partition_id()
            table32 = table.bitcast(dt.float32)
            s_ap = table32[bass.DynSlice(pid * cfg.shard, cfg.NW * P),
                           cfg.s_tgt_off:cfg.s_tgt_off + H]
            s_ap = s_ap.rearrange("(w p) h -> p w h", p=P)
            s_all = cpool.tile([P, cfg.NW, H], dt.float32)
            nc.sync.dma_start(out=s_all[:], in_=s_ap)
            s_hilo = cpool.tile([P, cfg.NW, 2, H], dt.bfloat16)
            s_hi32 = cpool.tile([P, cfg.NW, H], dt.float32)
            nc.vector.tensor_copy(out=s_hilo[:, :, 0, :], in_=s_all[:])
            nc.vector.tensor_copy(out=s_hi32[:], in_=s_hilo[:, :, 0, :])
            nc.vector.tensor_tensor(out=s_hilo[:, :, 1, :], in0=s_all[:],
                                    in1=s_hi32[:], op=mybir.AluOpType.subtract)

            # ---------------- phase 2: windows ----------------
            with tc.tile_pool(name="p2", bufs=2) as pool, \
                 tc.tile_pool(name="p2ps", bufs=2, space="PSUM") as pps:
                for w in range(cfg.NW):
                    Tw = sched.TW[w]
                    g1t = pool.tile([P, sched.TWmax, cfg.row_bf], dt.bfloat16, tag="g1t")
                    for (b, slot_off, nslots, idxcol0) in sched.calls[w]:
                        rows = min(cfg.NPAD, (b + 1) * cfg.bucket) - b * cfg.bucket
                        nc.gpsimd.dma_gather(
                            g1t[:, slot_off // P:(slot_off + nslots) // P, :],
                            table[b * cfg.bucket:b * cfg.bucket + rows, :],
                            g1i_sb[:, idxcol0:idxcol0 + nslots // 16],
                            nslots, nslots, cfg.row_bf)
                    jb = sched.tile_base[w]
                    oht = pool.tile([P, sched.TWmax, 2, P], oh_mdt, tag="oht")
                    nc.sync.dma_start(out=oht[:, :Tw, :, :], in_=ohd[:, jb:jb + Tw, :, :])

                    # s_tgt expansion (per tile) via transposed one-hot matmul
                    stgt = pps.tile([P, sched.TWmax, 2, H], dt.float32, tag="stgt")
                    for t in range(Tw):
                        nc.tensor.matmul(stgt[:, t, :, :], oht[:, t, 1, :],
                                         s_hilo[:, w, :, :], start=True, stop=True)
                    g1t32 = g1t.bitcast(dt.float32)
                    s_sum = pool.tile([P, sched.TWmax, H], dt.float32, tag="s_sum")
                    s_act = pool.tile([P, sched.TWmax, H], dt.float32, tag="s_act")
                    nc.vector.tensor_tensor(
                        out=s_sum[:, :Tw, :], in0=stgt[:, :Tw, 0, :],
                        in1=g1t32[:, :Tw, cfg.s_src_off:cfg.s_src_off + H],
                        op=mybir.AluOpType.add)
                    nc.vector.tensor_tensor(
                        out=s_act[:, :Tw, :], in0=stgt[:, :Tw, 1, :],
                        in1=s_sum[:, :Tw, :], op=mybir.AluOpType.add)
                    nc.vector.scalar_tensor_tensor(
                        out=s_sum[:, :Tw, :], in0=s_act[:, :Tw, :], scalar=cfg.leak,
                        in1=s_act[:, :Tw, :], op0=mybir.AluOpType.mult,
                        op1=mybir.AluOpType.max)
                    exp_t = pool.tile([P, sched.TWmax, H], dt.bfloat16, tag="exp_t")
                    nc.scalar.activation(out=exp_t[:, :Tw, :], in_=s_sum[:, :Tw, :],
                                         func=mybir.ActivationFunctionType.Exp)

                    w_t = pool.tile([P, sched.TWmax, HID], dt.bfloat16, tag="w_t")
                    proj4 = g1t[:, :Tw, 0:HID].rearrange("p t (h f) -> p t h f", h=H)
                    exp4 = _bcast_last(exp_t[:, :Tw, :], cfg.F)
                    out4 = w_t[:, :Tw, :].rearrange("p t (h f) -> p t h f", h=H)
                    nc.vector.tensor_tensor(out=out4, in0=proj4, in1=exp4,
                                            op=mybir.AluOpType.mult)

                    agg = pps.tile([P, HID], dt.float32, tag="agg")
                    den = pps.tile([P, H], dt.float32, tag="den")
                    for t in range(Tw):
                        nc.tensor.matmul(agg[:], oht[:, t, 0, :], w_t[:, t, :],
                                         start=(t == 0), stop=(t == Tw - 1))
                        nc.tensor.matmul(den[:], oht[:, t, 0, :], exp_t[:, t, :],
                                         start=(t == 0), stop=(t == Tw - 1))

                    # flush: softmax divide + PReLU
                    den_sb = pool.tile([P, H, 1], dt.float32, tag="den_sb")
                    recip = pool.tile([P, H, 1], dt.float32, tag="recip")
                    nc.vector.tensor_scalar_add(out=den_sb[:, :, 0], in0=den[:],
                                                scalar1=1e-16)
                    nc.vector.reciprocal(out=recip[:], in_=den_sb[:])
                    z = pool.tile([P, HID], dt.float32, tag="z")
                    agg4 = agg[:].rearrange("p (h f) -> p h f", h=H)
                    z4 = z[:].rearrange("p (h f) -> p h f", h=H)
                    nc.vector.tensor_tensor(out=z4, in0=agg4,
                                            in1=_bcast_last(recip[:, :, 0], cfg.F),
                                            op=mybir.AluOpType.mult)
                    res = pool.tile([P, HID], dt.float32, tag="res")
                    nc.vector.scalar_tensor_tensor(
                        out=res[:], in0=z[:], scalar=a_sb[:, 0:1], in1=z[:],
                        op0=mybir.AluOpType.mult, op1=mybir.AluOpType.max)
                    nc.sync.dma_start(out=out[w * P:(w + 1) * P, :], in_=res[:])

    nc.compile()
    return nc


def prepare(cfg: Cfg, inputs):
    """Host-side prep shared by HW and sim paths.

    Returns (sched, in_maps, assemble) where assemble(core_outs) -> full out.
    """
    X = np.asarray(inputs["in_nodes_features"], np.float32)
    ei = np.asarray(inputs["edge_index"], np.int64)
    W = np.asarray(inputs["W"], np.float32)
    b_lin = np.asarray(inputs["b_lin"], np.float32)
    a_src = np.asarray(inputs["a_src"], np.float32)
    a_tgt = np.asarray(inputs["a_tgt"], np.float32)
    bias = np.asarray(inputs["bias"], np.float32)
    prelu_a = float(np.asarray(inputs["prelu_a"], np.float32))

    assert np.all(b_lin == 0) and np.all(bias == 0), "nonzero bias unsupported"
    assert 0.0 <= prelu_a <= 1.0, "prelu_a outside [0,1] unsupported"

    src, trg = ei[0], ei[1]
    core_of = trg // cfg.shard
    win_of = (trg % cfg.shard) // P
    buck_of = src // cfg.bucket
    counts = np.zeros((cfg.ncores, cfg.NW, cfg.NB), np.int64)
    for k in range(cfg.ncores):
        m = core_of == k
        counts[k] = np.bincount(
            win_of[m] * cfg.NB + buck_of[m],
            minlength=cfg.NW * cfg.NB).reshape(cfg.NW, cfg.NB)
    sched = build_schedule(cfg, counts)

    xt = pack_xt(cfg, X)
    wtp, wap = pack_w(cfg, W, a_src, a_tgt)
    av = np.full((P, 1), prelu_a, np.float32)

    in_maps = []
    for k in range(cfg.ncores):
        g1i_k, oh_k = prep_core(cfg, sched, src, trg, k)
        in_maps.append({
            "xt": xt, "wt": wtp, "wa": wap,
            "g1i": g1i_k, "ohd": oh_k, "avec": av,
        })

    def assemble(core_outs):
        return np.concatenate(
            [np.asarray(o["out"][: cfg.shard], np.float32) for o in core_outs], axis=0)

    return sched, in_maps, assemble


_BUILT = {}


def _get_built(cfg: Cfg, sched: Schedule):
    key = (cfg.N, cfg.E, cfg.HID, cfg.HEADS, cfg.ncores, cfg.bucket,
           tuple(sched.TW), sched.idxcols)
    if key not in _BUILT:
        _BUILT[key] = build_nc(cfg, sched)
    return _BUILT[key]


def kernel(**inputs):
    from concourse.bass_utils import run_bass_kernel_spmd

    cfg = Cfg()
    sched, in_maps, assemble = prepare(cfg, inputs)
    nc = _get_built(cfg, sched)
    res = run_bass_kernel_spmd(nc, in_maps, core_ids=list(range(cfg.ncores)))
    return assemble(res.results)



# revision 14
# speedup vs baseline: 5.6882x; 5.6882x over previous
"""GAT layer (multi-head graph attention) on 8 TRN2 NeuronCores.

Strategy (per sharding hint): destination nodes are sharded across the 8
cores.  Each core:
  phase 1: computes the full projection table redundantly (bf16 GEMM
           X @ W.T plus the per-head attention score reductions), packed
           as [proj bf16 | s_src f32 | s_tgt f32 | pad] rows in local HBM.
  phase 2: walks its shard's destination windows (128 targets / window).
           Edges are pre-sorted by (window, src-bucket) on the host;
           dma_gather pulls the source rows (int16 indices per 32768-row
           bucket), scores -> leaky-relu -> exp run batched per window,
           and one-hot matmuls (host-streamed) accumulate both the
           softmax denominator and the weighted aggregation in PSUM.
           Softmax division + PReLU happen once per window at flush.

kernel(**inputs) takes the FULL inputs and returns the FULL output.
"""

import math
from dataclasses import dataclass, field

import numpy as np
import ml_dtypes

BF16 = ml_dtypes.bfloat16
P = 128


def _ceil(a, b):
    return -(-a // b)


@dataclass
class Cfg:
    N: int = 100000
    E: int = 800000
    HID: int = 512
    HEADS: int = 8
    ncores: int = 8
    bucket: int = 32768
    leak: float = 0.01
    oh_bf16: bool = False  # one-hot stream dtype (fp8 halves DMA; bf16 fallback)
    out_bf16: bool = True  # device writes bf16 output (cast to f32 on host)
    compact: bool = True   # per-core table holds only shard + edge-source rows

    def __post_init__(self):
        assert self.N % self.ncores == 0
        assert self.bucket <= 32768
        self.F = self.HID // self.HEADS
        self.shard = self.N // self.ncores
        self.NW = _ceil(self.shard, P)          # windows per core
        self.NB = _ceil(self.N, self.bucket)    # src buckets (int16 range)
        self.NT = _ceil(self.N, P)              # projection tiles
        self.NPAD = self.NT * P

    def set_table_rows(self, nrows: int):
        """Shrink the projection table to nrows (compacted per-core order)."""
        self.NT = _ceil(nrows, P)
        self.NPAD = self.NT * P
        self.NB = _ceil(self.NPAD, self.bucket)
        self.KP = min(self.HID, P)              # contraction partitions
        self.KT = self.HID // self.KP           # contraction tiles
        row_bytes = self.HID * 2 + 2 * self.HEADS * 4
        self.row_bytes = _ceil(row_bytes, 256) * 256
        self.row_bf = self.row_bytes // 2
        self.row_f32 = self.row_bytes // 4
        self.s_src_off = self.HID // 2          # f32 col of s_src in a row
        self.s_tgt_off = self.HID // 2 + self.HEADS


@dataclass
class Schedule:
    """Core-independent (uniform) phase-2 schedule."""
    seg: np.ndarray          # [NW, NB] slot counts (128-aligned, global max)
    TW: list                 # tiles per window
    TWmax: int
    calls: list              # per window: list of (b, slot_off, nslots, idxcol0)
    idxcols: int             # total int16 idx columns (per 16-wrap row)
    TT: int                  # total tiles
    tile_base: list          # first global tile index of each window


def build_schedule(cfg: Cfg, counts: np.ndarray) -> Schedule:
    """counts: [ncores, NW, NB] edge counts."""
    maxcnt = counts.max(axis=0)  # [NW, NB]
    seg = np.where(maxcnt > 0, _ceil(maxcnt, P) * P, 0).astype(np.int64)
    TW, calls, tile_base = [], [], []
    idxcol = 0
    tt = 0
    for w in range(cfg.NW):
        tile_base.append(tt)
        wcalls = []
        off = 0
        for b in range(cfg.NB):
            s = int(seg[w, b])
            if s == 0:
                continue
            # gather only the real rows (16-rounded); trailing slots keep
            # stale SBUF (one-time memset makes that finite -> OH col is 0)
            nreal = min(s, _ceil(int(maxcnt[w, b]), 16) * 16)
            wcalls.append((b, off, s, idxcol, nreal))
            off += s
            idxcol += s // 16
        assert off % P == 0
        TW.append(off // P)
        tt += off // P
        calls.append(wcalls)
    return Schedule(seg=seg, TW=TW, TWmax=max(TW), calls=calls,
                    idxcols=idxcol, TT=tt, tile_base=tile_base)


def prep_core(cfg: Cfg, sched: Schedule, esrc, trel):
    """Per-core input arrays: g1 idx stream and one-hot stream.

    esrc: per-edge source row in this core's (possibly compacted) table.
    trel: per-edge target relative to the core's shard base.
    """
    oh_dt = BF16 if cfg.oh_bf16 else ml_dtypes.float8_e4m3
    win = trel // P
    buck = esrc // cfg.bucket
    # order edges by (window, bucket); stable so host/device agree
    order = np.lexsort((buck, win))
    esrc, trel, win, buck = (a[order] for a in (esrc, trel, win, buck))

    g1i = np.zeros((P, sched.idxcols), np.int16)
    oh = np.zeros((P, sched.TT, 2, P), oh_dt)

    # per (window, bucket) segment boundaries
    key = win * cfg.NB + buck
    # edge ranges per (w, b)
    starts = np.searchsorted(key, np.arange(cfg.NW * cfg.NB), side="left")
    ends = np.searchsorted(key, np.arange(cfg.NW * cfg.NB), side="right")

    for w in range(cfg.NW):
        for (b, slot_off, nslots, idxcol0, _nreal) in sched.calls[w]:
            lo, hi = int(starts[w * cfg.NB + b]), int(ends[w * cfg.NB + b])
            cnt = hi - lo
            assert cnt <= nslots
            idx = np.zeros(nslots, np.int16)
            idx[:cnt] = (esrc[lo:hi] - b * cfg.bucket).astype(np.int16)
            blk = idx.reshape(nslots // 16, 16).T          # [16, cols]
            g1i[:, idxcol0:idxcol0 + nslots // 16] = np.tile(blk, (8, 1))
            # one-hots for this segment's tiles
            tloc = (trel[lo:hi] - w * P).astype(np.int64)  # [cnt] in [0,128)
            t0 = sched.tile_base[w] + slot_off // P
            for j in range(nslots // P):
                s0, s1 = j * P, min((j + 1) * P, cnt)
                if s1 <= s0:
                    continue
                rows = np.arange(s0, s1) - s0
                cols = tloc[s0:s1]
                oh[rows, t0 + j, 0, cols] = oh_dt(1.0)
                oh[cols, t0 + j, 1, rows] = oh_dt(1.0)
    return g1i, oh


def pack_xt(cfg: Cfg, X: np.ndarray) -> np.ndarray:
    """X [N, HID] f32 -> bf16 packed [KP, NT, KT, P]: (p, j, ki, n) = X[j*P+n, ki*KP+p]."""
    Xp = np.zeros((cfg.NPAD, cfg.HID), np.float32)
    Xp[: X.shape[0]] = np.asarray(X, np.float32)
    Xb = Xp.astype(BF16)
    # [NT, P(n), KT, KP(p)] -> transpose to [KP, NT, KT, P]
    v = Xb.reshape(cfg.NT, P, cfg.KT, cfg.KP)
    return np.ascontiguousarray(v.transpose(3, 0, 2, 1))


def pack_w(cfg: Cfg, W, a_src, a_tgt):
    """Returns wt [KP, KT, HID] bf16 and wa [KP, KT, 2*HEADS] bf16."""
    WT = W.T.astype(np.float32)                       # [HID(d), HID(o)]
    wa_s = (W.reshape(cfg.HEADS, cfg.F, cfg.HID)
            * np.asarray(a_src, np.float32).reshape(cfg.HEADS, cfg.F, 1)).sum(1)  # [H, d]
    wa_t = (W.reshape(cfg.HEADS, cfg.F, cfg.HID)
            * np.asarray(a_tgt, np.float32).reshape(cfg.HEADS, cfg.F, 1)).sum(1)
    WA = np.concatenate([wa_s.T, wa_t.T], axis=1)     # [d, 2H]
    wt = np.ascontiguousarray(
        WT.astype(BF16).reshape(cfg.KT, cfg.KP, cfg.HID).transpose(1, 0, 2))
    wa = np.ascontiguousarray(
        WA.astype(BF16).reshape(cfg.KT, cfg.KP, 2 * cfg.HEADS).transpose(1, 0, 2))
    return wt, wa


def _bcast_last(ap, n):
    """Append a 0-stride broadcast dim of size n to an AP."""
    import concourse.bass as bass
    lst = [list(x) for x in ap.ap] + [[0, n]]
    return bass.AP(ap.tensor, ap.offset, lst)


def build_nc(cfg: Cfg, sched: Schedule, phases: str = "full"):
    import concourse.bacc as bacc
    import concourse.bass as bass
    import concourse.mybir as mybir
    from concourse.tile import TileContext

    dt = mybir.dt
    oh_mdt = dt.bfloat16 if cfg.oh_bf16 else dt.float8e4
    out_mdt = dt.bfloat16 if cfg.out_bf16 else dt.float32
    H, HID, KT, KP = cfg.HEADS, cfg.HID, cfg.KT, cfg.KP

    nc = bacc.Bacc("TRN2", target_bir_lowering=False)

    xt = nc.dram_tensor("xt", [KP, cfg.NT, KT, P], dt.bfloat16, kind="ExternalInput")
    wt = nc.dram_tensor("wt", [KP, KT, HID], dt.bfloat16, kind="ExternalInput")
    wa = nc.dram_tensor("wa", [KP, KT, 2 * H], dt.bfloat16, kind="ExternalInput")
    g1i = nc.dram_tensor("g1i", [P, sched.idxcols], dt.int16, kind="ExternalInput")
    ohd = nc.dram_tensor("ohd", [P, sched.TT, 2, P], oh_mdt, kind="ExternalInput")
    avec = nc.dram_tensor("avec", [P, 1], dt.float32, kind="ExternalInput")
    out = nc.dram_tensor("out", [cfg.NW * P, HID], out_mdt, kind="ExternalOutput")

    with TileContext(nc) as tc:
        with tc.tile_pool(name="const", bufs=1) as cpool, \
             tc.tile_pool(name="dram", bufs=1, space="DRAM") as dpool:
            table = dpool.tile([cfg.NPAD, cfg.row_bf], dt.bfloat16)
            wt_sb = cpool.tile([KP, KT, HID], dt.bfloat16)
            nc.sync.dma_start(out=wt_sb[:], in_=wt[:, :, :])
            wa_sb = cpool.tile([KP, KT, 2 * H], dt.bfloat16)
            nc.sync.dma_start(out=wa_sb[:], in_=wa[:, :, :])
            if phases == "full":
                a_sb = cpool.tile([P, 1], dt.float32)
                nc.sync.dma_start(out=a_sb[:], in_=avec[:, :])
            if phases in ("full", "p1g"):
                g1i_sb = cpool.tile([P, sched.idxcols], dt.int16)
                nc.sync.dma_start(out=g1i_sb[:], in_=g1i[:, :])

            # ---------------- phase 1: projection table ----------------
            with tc.tile_pool(name="p1", bufs=3) as xpool, \
                 tc.tile_pool(name="p1ps", bufs=2, space="PSUM") as pspool, \
                 tc.tile_pool(name="p1st", bufs=3) as stpool:
                for j in range(cfg.NT):
                    xtile = xpool.tile([KP, KT, P], dt.bfloat16, tag="x")
                    nc.sync.dma_start(out=xtile[:], in_=xt[:, j, :, :])
                    ps1 = pspool.tile([P, HID], dt.float32, tag="ps1")
                    ps2 = pspool.tile([P, 2 * H], dt.float32, tag="ps2")
                    for ki in range(KT):
                        nc.tensor.matmul(ps1[:], xtile[:, ki, :], wt_sb[:, ki, :],
                                         start=(ki == 0), stop=(ki == KT - 1))
                    for ki in range(KT):
                        nc.tensor.matmul(ps2[:], xtile[:, ki, :], wa_sb[:, ki, :],
                                         start=(ki == 0), stop=(ki == KT - 1))
                    stg = stpool.tile([P, cfg.row_bf], dt.bfloat16, tag="stg")
                    stg32 = stg.bitcast(dt.float32)
                    nc.scalar.copy(out=stg[:, 0:HID], in_=ps1[:])
                    nc.scalar.copy(out=stg32[:, cfg.s_src_off:cfg.s_src_off + 2 * H],
                                   in_=ps2[:])
                    if cfg.s_tgt_off + H < cfg.row_f32:
                        nc.vector.memset(stg32[:, cfg.s_tgt_off + H:cfg.row_f32], 0.0)
                    nc.sync.dma_start(out=table[j * P:(j + 1) * P, :], in_=stg[:])

            tc.strict_bb_all_engine_barrier()

            if phases == "p1":
                with tc.tile_pool(name="dbg", bufs=2) as dbgp:
                    for w in range(cfg.NW):
                        res = dbgp.tile([P, HID], out_mdt, tag="res")
                        nc.vector.memset(res[:], 0.0)
                        nc.sync.dma_start(out=out[w * P:(w + 1) * P, :], in_=res[:])
                nc.compile()
                return nc

            if phases == "p1g":
                with tc.tile_pool(name="dbg", bufs=2) as dbgp:
                    for w in range(cfg.NW):
                        g1t = dbgp.tile([P, sched.TWmax, cfg.row_bf], dt.bfloat16,
                                        tag="g1t")
                        for (b, slot_off, nslots, idxcol0, nreal) in sched.calls[w]:
                            rows = min(cfg.NPAD, (b + 1) * cfg.bucket) - b * cfg.bucket
                            t0, t1 = slot_off // P, slot_off // P + _ceil(nreal, P)
                            nc.gpsimd.dma_gather(
                                g1t[:, t0:t1, :],
                                table[b * cfg.bucket:b * cfg.bucket + rows, :],
                                g1i_sb[:, idxcol0:idxcol0 + nslots // 16],
                                nreal, nreal, cfg.row_bf)
                        res = dbgp.tile([P, HID], out_mdt, tag="res")
                        nc.vector.memset(res[:], 0.0)
                        nc.sync.dma_start(out=out[w * P:(w + 1) * P, :], in_=res[:])
                nc.compile()
                return nc

            # ---------------- phase 1.5: resident s_tgt (hi/lo bf16) ----------------
            table32 = table.bitcast(dt.float32)
            if cfg.compact:
                s_ap = table32[0:cfg.NW * P, cfg.s_tgt_off:cfg.s_tgt_off + H]
            else:
                pid = nc.sync.partition_id()
                s_ap = table32[bass.DynSlice(pid * cfg.shard, cfg.NW * P),
                               cfg.s_tgt_off:cfg.s_tgt_off + H]
            s_ap = s_ap.rearrange("(w p) h -> p w h", p=P)
            s_all = cpool.tile([P, cfg.NW, H], dt.float32)
            nc.sync.dma_start(out=s_all[:], in_=s_ap)
            s_hilo = cpool.tile([P, cfg.NW, 2, H], dt.bfloat16)
            s_hi32 = cpool.tile([P, cfg.NW, H], dt.float32)
            nc.vector.tensor_copy(out=s_hilo[:, :, 0, :], in_=s_all[:])
            nc.vector.tensor_copy(out=s_hi32[:], in_=s_hilo[:, :, 0, :])
            nc.vector.tensor_tensor(out=s_hilo[:, :, 1, :], in0=s_all[:],
                                    in1=s_hi32[:], op=mybir.AluOpType.subtract)

            # ---------------- phase 2: windows ----------------
            with tc.tile_pool(name="p2", bufs=2) as pool, \
                 tc.tile_pool(name="p2ps", bufs=2, space="PSUM") as pps:
                # one-time memset of the gather slots: trailing (pad) slots are
                # never gathered again, and garbage bits could be NaN/Inf and
                # poison 0*NaN in the aggregation matmuls.
                g1bufs = []
                for _ in range(2):
                    g1t = pool.tile([P, sched.TWmax, cfg.row_bf], dt.bfloat16,
                                    tag="g1t")
                    nc.vector.memset(g1t.bitcast(dt.float32)[:], 0.0)
                    g1bufs.append(g1t)
                for w in range(cfg.NW):
                    Tw = sched.TW[w]
                    g1t = pool.tile([P, sched.TWmax, cfg.row_bf], dt.bfloat16, tag="g1t")
                    for (b, slot_off, nslots, idxcol0, nreal) in sched.calls[w]:
                        rows = min(cfg.NPAD, (b + 1) * cfg.bucket) - b * cfg.bucket
                        t0, t1 = slot_off // P, slot_off // P + _ceil(nreal, P)
                        nc.gpsimd.dma_gather(
                            g1t[:, t0:t1, :],
                            table[b * cfg.bucket:b * cfg.bucket + rows, :],
                            g1i_sb[:, idxcol0:idxcol0 + nslots // 16],
                            nreal, nreal, cfg.row_bf)
                    jb = sched.tile_base[w]
                    oht = pool.tile([P, sched.TWmax, 2, P], oh_mdt, tag="oht")
                    nc.sync.dma_start(out=oht[:, :Tw, :, :], in_=ohd[:, jb:jb + Tw, :, :])

                    # s_tgt expansion (per tile) via transposed one-hot matmul
                    stgt = pps.tile([P, sched.TWmax, 2, H], dt.float32, tag="stgt")
                    for t in range(Tw):
                        nc.tensor.matmul(stgt[:, t, :, :], oht[:, t, 1, :],
                                         s_hilo[:, w, :, :], start=True, stop=True)
                    g1t32 = g1t.bitcast(dt.float32)
                    s_sum = pool.tile([P, sched.TWmax, H], dt.float32, tag="s_sum")
                    s_act = pool.tile([P, sched.TWmax, H], dt.float32, tag="s_act")
                    nc.vector.tensor_tensor(
                        out=s_sum[:, :Tw, :], in0=stgt[:, :Tw, 0, :],
                        in1=g1t32[:, :Tw, cfg.s_src_off:cfg.s_src_off + H],
                        op=mybir.AluOpType.add)
                    nc.vector.tensor_tensor(
                        out=s_act[:, :Tw, :], in0=stgt[:, :Tw, 1, :],
                        in1=s_sum[:, :Tw, :], op=mybir.AluOpType.add)
                    nc.vector.scalar_tensor_tensor(
                        out=s_sum[:, :Tw, :], in0=s_act[:, :Tw, :], scalar=cfg.leak,
                        in1=s_act[:, :Tw, :], op0=mybir.AluOpType.mult,
                        op1=mybir.AluOpType.max)
                    # exp, expanded to F copies per head on ScalarE (0-stride
                    # read) so the big multiply below runs in DVE 2x mode
                    exp_r = pool.tile([P, sched.TWmax, HID], dt.bfloat16, tag="exp_r")
                    exp_in = _bcast_last(s_sum[:, :Tw, :], cfg.F)
                    nc.scalar.activation(out=exp_r[:, :Tw, :].rearrange(
                                             "p t (h f) -> p t h f", h=H),
                                         in_=exp_in,
                                         func=mybir.ActivationFunctionType.Exp)

                    w_t = pool.tile([P, sched.TWmax, HID], dt.bfloat16, tag="w_t")
                    nc.vector.tensor_tensor(out=w_t[:, :Tw, :],
                                            in0=g1t[:, :Tw, 0:HID],
                                            in1=exp_r[:, :Tw, :],
                                            op=mybir.AluOpType.mult)

                    exp_h = exp_r[:].rearrange("p t (h f) -> p t h f", h=H)
                    agg = pps.tile([P, HID], dt.float32, tag="agg")
                    den = pps.tile([P, H], dt.float32, tag="den")
                    for t in range(Tw):
                        nc.tensor.matmul(agg[:], oht[:, t, 0, :], w_t[:, t, :],
                                         start=(t == 0), stop=(t == Tw - 1))
                        nc.tensor.matmul(den[:], oht[:, t, 0, :], exp_h[:, t, :, 0],
                                         start=(t == 0), stop=(t == Tw - 1))

                    # flush: softmax divide + PReLU
                    den_sb = pool.tile([P, H, 1], dt.float32, tag="den_sb")
                    recip = pool.tile([P, H, 1], dt.float32, tag="recip")
                    nc.vector.tensor_scalar_add(out=den_sb[:, :, 0], in0=den[:],
                                                scalar1=1e-16)
                    nc.vector.reciprocal(out=recip[:], in_=den_sb[:])
                    z = pool.tile([P, HID], dt.float32, tag="z")
                    agg4 = agg[:].rearrange("p (h f) -> p h f", h=H)
                    z4 = z[:].rearrange("p (h f) -> p h f", h=H)
                    nc.vector.tensor_tensor(out=z4, in0=agg4,
                                            in1=_bcast_last(recip[:, :, 0], cfg.F),
                                            op=mybir.AluOpType.mult)
                    res = pool.tile([P, HID], out_mdt, tag="res")
                    nc.vector.scalar_tensor_tensor(
                        out=res[:], in0=z[:], scalar=a_sb[:, 0:1], in1=z[:],
                        op0=mybir.AluOpType.mult, op1=mybir.AluOpType.max)
                    nc.sync.dma_start(out=out[w * P:(w + 1) * P, :], in_=res[:])

    nc.compile()
    return nc


def prepare(cfg: Cfg, inputs):
    """Host-side prep shared by HW and sim paths.

    Returns (sched, in_maps, assemble) where assemble(core_outs) -> full out.
    """
    X = np.asarray(inputs["in_nodes_features"], np.float32)
    ei = np.asarray(inputs["edge_index"], np.int64)
    W = np.asarray(inputs["W"], np.float32)
    b_lin = np.asarray(inputs["b_lin"], np.float32)
    a_src = np.asarray(inputs["a_src"], np.float32)
    a_tgt = np.asarray(inputs["a_tgt"], np.float32)
    bias = np.asarray(inputs["bias"], np.float32)
    prelu_a = float(np.asarray(inputs["prelu_a"], np.float32))

    assert np.all(b_lin == 0) and np.all(bias == 0), "nonzero bias unsupported"
    assert 0.0 <= prelu_a <= 1.0, "prelu_a outside [0,1] unsupported"

    src, trg = ei[0], ei[1]
    core_of = trg // cfg.shard

    # per-core edge lists (+ optional table compaction: shard rows first,
    # then the core's out-of-shard edge sources)
    core_esrc, core_trel, core_nodes = [], [], []
    for k in range(cfg.ncores):
        m = core_of == k
        esrc_k = src[m]
        trel_k = trg[m] - k * cfg.shard
        if cfg.compact:
            lo, hi = k * cfg.shard, (k + 1) * cfg.shard
            ext = np.unique(esrc_k[(esrc_k < lo) | (esrc_k >= hi)])
            nodes_k = np.concatenate([np.arange(lo, hi), ext])
            remap = np.empty(cfg.N, np.int64)
            remap[nodes_k] = np.arange(len(nodes_k))
            esrc_k = remap[esrc_k]
        else:
            nodes_k = None
        core_esrc.append(esrc_k)
        core_trel.append(trel_k)
        core_nodes.append(nodes_k)

    if cfg.compact:
        cfg.set_table_rows(max(len(n) for n in core_nodes))

    counts = np.zeros((cfg.ncores, cfg.NW, cfg.NB), np.int64)
    for k in range(cfg.ncores):
        counts[k] = np.bincount(
            (core_trel[k] // P) * cfg.NB + core_esrc[k] // cfg.bucket,
            minlength=cfg.NW * cfg.NB).reshape(cfg.NW, cfg.NB)
    sched = build_schedule(cfg, counts)

    wtp, wap = pack_w(cfg, W, a_src, a_tgt)
    av = np.full((P, 1), prelu_a, np.float32)
    if not cfg.compact:
        xt = pack_xt(cfg, X)

    in_maps = []
    for k in range(cfg.ncores):
        g1i_k, oh_k = prep_core(cfg, sched, core_esrc[k], core_trel[k])
        xt_k = pack_xt(cfg, X[core_nodes[k]]) if cfg.compact else xt
        in_maps.append({
            "xt": xt_k, "wt": wtp, "wa": wap,
            "g1i": g1i_k, "ohd": oh_k, "avec": av,
        })

    def assemble(core_outs):
        return np.concatenate(
            [np.asarray(o["out"][: cfg.shard], np.float32) for o in core_outs], axis=0)

    return sched, in_maps, assemble


_BUILT = {}


def _get_built(cfg: Cfg, sched: Schedule):
    key = (cfg.N, cfg.E, cfg.HID, cfg.HEADS, cfg.ncores, cfg.bucket,
           tuple(sched.TW), sched.idxcols)
    if key not in _BUILT:
        _BUILT[key] = build_nc(cfg, sched)
    return _BUILT[key]


def kernel(**inputs):
    from concourse.bass_utils import run_bass_kernel_spmd

    cfg = Cfg()
    sched, in_maps, assemble = prepare(cfg, inputs)
    nc = _get_built(cfg, sched)
    res = run_bass_kernel_spmd(nc, in_maps, core_ids=list(range(cfg.ncores)))
    return assemble(res.results)



# revision 29
# speedup vs baseline: 6.3034x; 1.1082x over previous
"""GAT layer (multi-head graph attention) on 8 TRN2 NeuronCores.

Strategy (per sharding hint): destination nodes are sharded across the 8
cores.  Each core:
  phase 1: computes the full projection table redundantly (bf16 GEMM
           X @ W.T plus the per-head attention score reductions), packed
           as [proj bf16 | s_src f32 | s_tgt f32 | pad] rows in local HBM.
  phase 2: walks its shard's destination windows (128 targets / window).
           Edges are pre-sorted by (window, src-bucket) on the host;
           dma_gather pulls the source rows (int16 indices per 32768-row
           bucket), scores -> leaky-relu -> exp run batched per window,
           and one-hot matmuls (host-streamed) accumulate both the
           softmax denominator and the weighted aggregation in PSUM.
           Softmax division + PReLU happen once per window at flush.

kernel(**inputs) takes the FULL inputs and returns the FULL output.
"""

import math
from dataclasses import dataclass, field

import numpy as np
import ml_dtypes

BF16 = ml_dtypes.bfloat16
P = 128


def _ceil(a, b):
    return -(-a // b)


@dataclass
class Cfg:
    N: int = 100000
    E: int = 800000
    HID: int = 512
    HEADS: int = 8
    ncores: int = 8
    bucket: int = 32768
    leak: float = 0.01
    oh_bf16: bool = False  # one-hot stream dtype (fp8 halves DMA; bf16 fallback)
    out_bf16: bool = True  # device writes bf16 output (cast to f32 on host)
    compact: bool = True   # per-core table holds only shard + edge-source rows

    def __post_init__(self):
        assert self.N % self.ncores == 0
        assert self.bucket <= 32768
        self.F = self.HID // self.HEADS
        self.shard = self.N // self.ncores
        self.NW = _ceil(self.shard, P)          # windows per core
        self.NB = _ceil(self.N, self.bucket)    # src buckets (int16 range)
        self.NT = _ceil(self.N, P)              # projection tiles
        self.NPAD = self.NT * P

        self.KP = min(self.HID, P)              # contraction partitions
        self.KT = self.HID // self.KP           # contraction tiles
        row_bytes = self.HID * 2 + 2 * self.HEADS * 4
        self.row_bytes = _ceil(row_bytes, 256) * 256
        self.row_bf = self.row_bytes // 2
        self.row_f32 = self.row_bytes // 4
        self.s_src_off = self.HID // 2          # f32 col of s_src in a row
        self.s_tgt_off = self.HID // 2 + self.HEADS

    def set_table_rows(self, nrows: int):
        """Shrink the projection table to nrows (compacted per-core order)."""
        self.NT = _ceil(nrows, P)
        self.NPAD = self.NT * P
        self.NB = _ceil(self.NPAD, self.bucket)


@dataclass
class Schedule:
    """Core-independent (uniform) phase-2 schedule."""
    seg: np.ndarray          # [NW, NB] slot counts (128-aligned, global max)
    TW: list                 # tiles per window
    TWmax: int
    calls: list              # per window: list of (b, slot_off, nslots, idxcol0)
    idxcols: int             # total int16 idx columns (per 16-wrap row)
    TT: int                  # total tiles
    tile_base: list          # first global tile index of each window


def build_schedule(cfg: Cfg, counts: np.ndarray) -> Schedule:
    """counts: [ncores, NW, NB] edge counts."""
    maxcnt = counts.max(axis=0)  # [NW, NB]
    seg = np.where(maxcnt > 0, _ceil(maxcnt, P) * P, 0).astype(np.int64)
    TW, calls, tile_base = [], [], []
    idxcol = 0
    tt = 0
    for w in range(cfg.NW):
        tile_base.append(tt)
        wcalls = []
        off = 0
        for b in range(cfg.NB):
            s = int(seg[w, b])
            if s == 0:
                continue
            # gather only the real rows (16-rounded); trailing slots keep
            # stale SBUF (one-time memset makes that finite -> OH col is 0)
            nreal = min(s, _ceil(int(maxcnt[w, b]), 16) * 16)
            wcalls.append((b, off, s, idxcol, nreal))
            off += s
            idxcol += s // 16
        assert off % P == 0
        TW.append(off // P)
        tt += off // P
        calls.append(wcalls)
    return Schedule(seg=seg, TW=TW, TWmax=max(TW), calls=calls,
                    idxcols=idxcol, TT=tt, tile_base=tile_base)


def prep_core(cfg: Cfg, sched: Schedule, esrc, trel):
    """Per-core input arrays: g1 idx stream and one-hot stream.

    esrc: per-edge source row in this core's (possibly compacted) table.
    trel: per-edge target relative to the core's shard base.
    """
    oh_dt = BF16 if cfg.oh_bf16 else ml_dtypes.float8_e4m3
    win = trel // P
    buck = esrc // cfg.bucket
    # order edges by (window, bucket, src): src-ascending within a segment
    # makes the gather's random HBM reads address-monotonic (row locality)
    order = np.lexsort((esrc, buck, win))
    esrc, trel, win, buck = (a[order] for a in (esrc, trel, win, buck))

    g1i = np.zeros((P, sched.idxcols), np.int16)
    oh = np.zeros((P, sched.TT, 2, P), oh_dt)

    # per (window, bucket) segment boundaries
    key = win * cfg.NB + buck
    # edge ranges per (w, b)
    starts = np.searchsorted(key, np.arange(cfg.NW * cfg.NB), side="left")
    ends = np.searchsorted(key, np.arange(cfg.NW * cfg.NB), side="right")

    for w in range(cfg.NW):
        for (b, slot_off, nslots, idxcol0, _nreal) in sched.calls[w]:
            lo, hi = int(starts[w * cfg.NB + b]), int(ends[w * cfg.NB + b])
            cnt = hi - lo
            assert cnt <= nslots
            idx = np.zeros(nslots, np.int16)
            idx[:cnt] = (esrc[lo:hi] - b * cfg.bucket).astype(np.int16)
            if 0 < cnt < nslots:
                # pad slots re-gather the last real row: the duplicate read
                # hits the open HBM row instead of a cold bucket-base row
                idx[cnt:] = idx[cnt - 1]
            blk = idx.reshape(nslots // 16, 16).T          # [16, cols]
            g1i[:, idxcol0:idxcol0 + nslots // 16] = np.tile(blk, (8, 1))
            # one-hots for this segment's tiles
            tloc = (trel[lo:hi] - w * P).astype(np.int64)  # [cnt] in [0,128)
            t0 = sched.tile_base[w] + slot_off // P
            for j in range(nslots // P):
                s0, s1 = j * P, min((j + 1) * P, cnt)
                if s1 <= s0:
                    continue
                rows = np.arange(s0, s1) - s0
                cols = tloc[s0:s1]
                oh[rows, t0 + j, 0, cols] = oh_dt(1.0)
                oh[cols, t0 + j, 1, rows] = oh_dt(1.0)
    return g1i, oh


XJ = 4  # projection tiles per xt DMA block


def pack_xt(cfg: Cfg, X: np.ndarray) -> np.ndarray:
    """X [N, HID] f32 -> bf16 packed [KP, NJ, KT, XJ*P]:
    (p, jb, ki, j*P+n) = X[(jb*XJ+j)*P+n, ki*KP+p]."""
    nj = _ceil(cfg.NT, XJ)
    Xp = np.zeros((nj * XJ * P, cfg.HID), np.float32)
    Xp[: X.shape[0]] = np.asarray(X, np.float32)
    Xb = Xp.astype(BF16)
    v = Xb.reshape(nj, XJ * P, cfg.KT, cfg.KP)
    return np.ascontiguousarray(v.transpose(3, 0, 2, 1))


def pack_w(cfg: Cfg, W, a_src, a_tgt):
    """Returns wt [KP, KT, HID] bf16 and wa [KP, KT, 2*HEADS] bf16."""
    WT = W.T.astype(np.float32)                       # [HID(d), HID(o)]
    wa_s = (W.reshape(cfg.HEADS, cfg.F, cfg.HID)
            * np.asarray(a_src, np.float32).reshape(cfg.HEADS, cfg.F, 1)).sum(1)  # [H, d]
    wa_t = (W.reshape(cfg.HEADS, cfg.F, cfg.HID)
            * np.asarray(a_tgt, np.float32).reshape(cfg.HEADS, cfg.F, 1)).sum(1)
    WA = np.concatenate([wa_s.T, wa_t.T], axis=1)     # [d, 2H]
    wt = np.ascontiguousarray(
        WT.astype(BF16).reshape(cfg.KT, cfg.KP, cfg.HID).transpose(1, 0, 2))
    wa = np.ascontiguousarray(
        WA.astype(BF16).reshape(cfg.KT, cfg.KP, 2 * cfg.HEADS).transpose(1, 0, 2))
    return wt, wa


def _bcast_last(ap, n):
    """Append a 0-stride broadcast dim of size n to an AP."""
    import concourse.bass as bass
    lst = [list(x) for x in ap.ap] + [[0, n]]
    return bass.AP(ap.tensor, ap.offset, lst)


def build_nc(cfg: Cfg, sched: Schedule, phases: str = "full"):
    import concourse.bacc as bacc
    import concourse.bass as bass
    import concourse.mybir as mybir
    from concourse.tile import TileContext

    dt = mybir.dt
    oh_mdt = dt.bfloat16 if cfg.oh_bf16 else dt.float8e4
    out_mdt = dt.bfloat16 if cfg.out_bf16 else dt.float32
    H, HID, KT, KP = cfg.HEADS, cfg.HID, cfg.KT, cfg.KP

    nc = bacc.Bacc("TRN2", target_bir_lowering=False)

    NJ = _ceil(cfg.NT, XJ)
    xt = nc.dram_tensor("xt", [KP, NJ, KT, XJ * P], dt.bfloat16,
                        kind="ExternalInput")
    wt = nc.dram_tensor("wt", [KP, KT, HID], dt.bfloat16, kind="ExternalInput")
    wa = nc.dram_tensor("wa", [KP, KT, 2 * H], dt.bfloat16, kind="ExternalInput")
    g1i = nc.dram_tensor("g1i", [P, sched.idxcols], dt.int16, kind="ExternalInput")
    ohd = nc.dram_tensor("ohd", [P, sched.TT, 2, P], oh_mdt, kind="ExternalInput")
    avec = nc.dram_tensor("avec", [P, 1], dt.float32, kind="ExternalInput")
    out = nc.dram_tensor("out", [cfg.NW * P, HID], out_mdt, kind="ExternalOutput")

    with TileContext(nc) as tc:
        with tc.tile_pool(name="const", bufs=1) as cpool, \
             tc.tile_pool(name="dram", bufs=1, space="DRAM") as dpool:
            table = dpool.tile([cfg.NPAD, cfg.row_bf], dt.bfloat16)
            wt_sb = cpool.tile([KP, KT, HID], dt.bfloat16)
            nc.sync.dma_start(out=wt_sb[:], in_=wt[:, :, :])
            wa_sb = cpool.tile([KP, KT, 2 * H], dt.bfloat16)
            nc.sync.dma_start(out=wa_sb[:], in_=wa[:, :, :])
            if phases == "full":
                a_sb = cpool.tile([P, 1], dt.float32)
                nc.sync.dma_start(out=a_sb[:], in_=avec[:, :])
            if phases in ("full", "p1g"):
                g1i_sb = cpool.tile([P, sched.idxcols], dt.int16)
                nc.sync.dma_start(out=g1i_sb[:], in_=g1i[:, :])

            # ---------------- phase 1: projection table ----------------
            # rows are written 1088B-dense (proj + s_src + s_tgt); the DRAM
            # row tail (1088:1280) stays uninitialized and is never read.
            row_w = cfg.s_tgt_off + H  # f32 cols actually written
            with tc.tile_pool(name="p1", bufs=2) as xpool, \
                 tc.tile_pool(name="p1ps", bufs=3, space="PSUM") as psp1, \
                 tc.tile_pool(name="p1ps2", bufs=2, space="PSUM") as psp2, \
                 tc.tile_pool(name="p1st", bufs=3) as stpool:
                SJ = 2  # staging tiles per table write
                for jb in range(NJ):
                    xtile = xpool.tile([KP, KT, XJ * P], dt.bfloat16, tag="x")
                    nc.sync.dma_start(out=xtile[:], in_=xt[:, jb, :, :])
                    for j in range(XJ):
                        ps1 = psp1.tile([P, HID], dt.float32, tag="ps1")
                        ps2 = psp2.tile([P, 2 * H], dt.float32, tag="ps2")
                        for ki in range(KT):
                            nc.tensor.matmul(ps1[:], xtile[:, ki, j * P:(j + 1) * P],
                                             wt_sb[:, ki, :],
                                             start=(ki == 0), stop=(ki == KT - 1))
                            nc.tensor.matmul(ps2[:], xtile[:, ki, j * P:(j + 1) * P],
                                             wa_sb[:, ki, :],
                                             start=(ki == 0), stop=(ki == KT - 1))
                        jj = jb * XJ + j
                        if jj % SJ == 0:
                            stg = stpool.tile([P, SJ, cfg.row_bf], dt.bfloat16,
                                              tag="stg")
                            stg32 = stg.bitcast(dt.float32)
                        sl = jj % SJ
                        nc.scalar.copy(out=stg[:, sl, 0:HID], in_=ps1[:])
                        nc.scalar.copy(
                            out=stg32[:, sl, cfg.s_src_off:cfg.s_src_off + 2 * H],
                            in_=ps2[:])
                        if sl == SJ - 1:
                            r0 = (jj - sl) * P
                            dst = table[r0:r0 + SJ * P, 0:2 * row_w].rearrange(
                                "(j p) c -> p j c", p=P)
                            nc.sync.dma_start(out=dst,
                                              in_=stg[:, :, 0:2 * row_w])

            tc.strict_bb_all_engine_barrier()

            if phases == "p1":
                with tc.tile_pool(name="dbg", bufs=2) as dbgp:
                    for w in range(cfg.NW):
                        res = dbgp.tile([P, HID], out_mdt, tag="res")
                        nc.vector.memset(res[:], 0.0)
                        nc.sync.dma_start(out=out[w * P:(w + 1) * P, :], in_=res[:])
                nc.compile()
                return nc

            if phases == "p1g":
                with tc.tile_pool(name="dbg", bufs=2) as dbgp:
                    for w in range(cfg.NW):
                        g1t = dbgp.tile([P, sched.TWmax, cfg.row_bf], dt.bfloat16,
                                        tag="g1t")
                        for (b, slot_off, nslots, idxcol0, nreal) in sched.calls[w]:
                            rows = min(cfg.NPAD, (b + 1) * cfg.bucket) - b * cfg.bucket
                            t0, t1 = slot_off // P, slot_off // P + _ceil(nreal, P)
                            nc.gpsimd.dma_gather(
                                g1t[:, t0:t1, :],
                                table[b * cfg.bucket:b * cfg.bucket + rows, :],
                                g1i_sb[:, idxcol0:idxcol0 + nslots // 16],
                                nreal, nreal, cfg.row_bf)
                        res = dbgp.tile([P, HID], out_mdt, tag="res")
                        nc.vector.memset(res[:], 0.0)
                        nc.sync.dma_start(out=out[w * P:(w + 1) * P, :], in_=res[:])
                nc.compile()
                return nc

            # ---------------- phase 1.5: resident s_tgt (hi/lo bf16) ----------------
            table32 = table.bitcast(dt.float32)
            if cfg.compact:
                s_ap = table32[0:cfg.NW * P, cfg.s_tgt_off:cfg.s_tgt_off + H]
            else:
                pid = nc.sync.partition_id()
                s_ap = table32[bass.DynSlice(pid * cfg.shard, cfg.NW * P),
                               cfg.s_tgt_off:cfg.s_tgt_off + H]
            s_ap = s_ap.rearrange("(w p) h -> p w h", p=P)
            s_all = cpool.tile([P, cfg.NW, H], dt.float32)
            nc.sync.dma_start(out=s_all[:], in_=s_ap)
            s_hilo = cpool.tile([P, cfg.NW, 2, H], dt.bfloat16)
            s_hi32 = cpool.tile([P, cfg.NW, H], dt.float32)
            nc.vector.tensor_copy(out=s_hilo[:, :, 0, :], in_=s_all[:])
            nc.vector.tensor_copy(out=s_hi32[:], in_=s_hilo[:, :, 0, :])
            nc.vector.tensor_tensor(out=s_hilo[:, :, 1, :], in0=s_all[:],
                                    in1=s_hi32[:], op=mybir.AluOpType.subtract)

            # ---------------- phase 2: windows ----------------
            G1B = 3
            with tc.tile_pool(name="p2", bufs=2) as pool, \
                 tc.tile_pool(name="p2g", bufs=G1B) as gpool, \
                 tc.tile_pool(name="p2ps", bufs=2, space="PSUM") as pps:
                # one-time memset of the gather slots: trailing (pad) slots are
                # never gathered again, and garbage bits could be NaN/Inf and
                # poison 0*NaN in the aggregation matmuls.
                g1bufs = []
                for _ in range(G1B):
                    g1t = gpool.tile([P, sched.TWmax, cfg.row_bf], dt.bfloat16,
                                     tag="g1t")
                    nc.vector.memset(g1t.bitcast(dt.float32)[:], 0.0)
                    g1bufs.append(g1t)
                for w in range(cfg.NW):
                    Tw = sched.TW[w]
                    g1t = gpool.tile([P, sched.TWmax, cfg.row_bf], dt.bfloat16,
                                     tag="g1t")
                    for (b, slot_off, nslots, idxcol0, nreal) in sched.calls[w]:
                        rows = min(cfg.NPAD, (b + 1) * cfg.bucket) - b * cfg.bucket
                        t0, t1 = slot_off // P, slot_off // P + _ceil(nreal, P)
                        nc.gpsimd.dma_gather(
                            g1t[:, t0:t1, :],
                            table[b * cfg.bucket:b * cfg.bucket + rows, :],
                            g1i_sb[:, idxcol0:idxcol0 + nslots // 16],
                            nreal, nreal, cfg.row_bf)
                    jb = sched.tile_base[w]
                    oht = pool.tile([P, sched.TWmax, 2, P], oh_mdt, tag="oht")
                    nc.sync.dma_start(out=oht[:, :Tw, :, :], in_=ohd[:, jb:jb + Tw, :, :])

                    # s_tgt expansion (per tile) via transposed one-hot matmul
                    stgt = pps.tile([P, sched.TWmax, 2, H], dt.float32, tag="stgt")
                    for t in range(Tw):
                        nc.tensor.matmul(stgt[:, t, :, :], oht[:, t, 1, :],
                                         s_hilo[:, w, :, :], start=True, stop=True)
                    g1t32 = g1t.bitcast(dt.float32)
                    s_sum = pool.tile([P, sched.TWmax, H], dt.float32, tag="s_sum")
                    s_act = pool.tile([P, sched.TWmax, H], dt.float32, tag="s_act")
                    nc.vector.tensor_tensor(
                        out=s_sum[:, :Tw, :], in0=stgt[:, :Tw, 0, :],
                        in1=g1t32[:, :Tw, cfg.s_src_off:cfg.s_src_off + H],
                        op=mybir.AluOpType.add)
                    nc.vector.tensor_tensor(
                        out=s_act[:, :Tw, :], in0=stgt[:, :Tw, 1, :],
                        in1=s_sum[:, :Tw, :], op=mybir.AluOpType.add)
                    nc.vector.scalar_tensor_tensor(
                        out=s_sum[:, :Tw, :], in0=s_act[:, :Tw, :], scalar=cfg.leak,
                        in1=s_act[:, :Tw, :], op0=mybir.AluOpType.mult,
                        op1=mybir.AluOpType.max)
                    # exp, expanded to F copies per head on ScalarE (0-stride
                    # read) so the big multiply below runs in DVE 2x mode
                    exp_r = pool.tile([P, sched.TWmax, HID], dt.bfloat16, tag="exp_r")
                    exp_in = _bcast_last(s_sum[:, :Tw, :], cfg.F)
                    nc.scalar.activation(out=exp_r[:, :Tw, :].rearrange(
                                             "p t (h f) -> p t h f", h=H),
                                         in_=exp_in,
                                         func=mybir.ActivationFunctionType.Exp)

                    w_t = pool.tile([P, sched.TWmax, HID], dt.bfloat16, tag="w_t")
                    nc.vector.tensor_tensor(out=w_t[:, :Tw, :],
                                            in0=g1t[:, :Tw, 0:HID],
                                            in1=exp_r[:, :Tw, :],
                                            op=mybir.AluOpType.mult)

                    exp_h = exp_r[:].rearrange("p t (h f) -> p t h f", h=H)
                    agg = pps.tile([P, HID], dt.float32, tag="agg")
                    den = pps.tile([P, H], dt.float32, tag="den")
                    for t in range(Tw):
                        nc.tensor.matmul(agg[:], oht[:, t, 0, :], w_t[:, t, :],
                                         start=(t == 0), stop=(t == Tw - 1))
                        nc.tensor.matmul(den[:], oht[:, t, 0, :], exp_h[:, t, :, 0],
                                         start=(t == 0), stop=(t == Tw - 1))

                    # flush: softmax divide + PReLU
                    den_sb = pool.tile([P, H, 1], dt.float32, tag="den_sb")
                    recip = pool.tile([P, H, 1], dt.float32, tag="recip")
                    nc.vector.tensor_scalar_add(out=den_sb[:, :, 0], in0=den[:],
                                                scalar1=1e-16)
                    nc.vector.reciprocal(out=recip[:], in_=den_sb[:])
                    z = pool.tile([P, HID], dt.float32, tag="z")
                    agg4 = agg[:].rearrange("p (h f) -> p h f", h=H)
                    z4 = z[:].rearrange("p (h f) -> p h f", h=H)
                    nc.vector.tensor_tensor(out=z4, in0=agg4,
                                            in1=_bcast_last(recip[:, :, 0], cfg.F),
                                            op=mybir.AluOpType.mult)
                    res = pool.tile([P, HID], out_mdt, tag="res")
                    nc.vector.scalar_tensor_tensor(
                        out=res[:], in0=z[:], scalar=a_sb[:, 0:1], in1=z[:],
                        op0=mybir.AluOpType.mult, op1=mybir.AluOpType.max)
                    nc.sync.dma_start(out=out[w * P:(w + 1) * P, :], in_=res[:])

    nc.compile()
    return nc


def prepare(cfg: Cfg, inputs):
    """Host-side prep shared by HW and sim paths.

    Returns (sched, in_maps, assemble) where assemble(core_outs) -> full out.
    """
    X = np.asarray(inputs["in_nodes_features"], np.float32)
    ei = np.asarray(inputs["edge_index"], np.int64)
    W = np.asarray(inputs["W"], np.float32)
    b_lin = np.asarray(inputs["b_lin"], np.float32)
    a_src = np.asarray(inputs["a_src"], np.float32)
    a_tgt = np.asarray(inputs["a_tgt"], np.float32)
    bias = np.asarray(inputs["bias"], np.float32)
    prelu_a = float(np.asarray(inputs["prelu_a"], np.float32))

    assert np.all(b_lin == 0) and np.all(bias == 0), "nonzero bias unsupported"
    assert 0.0 <= prelu_a <= 1.0, "prelu_a outside [0,1] unsupported"

    src, trg = ei[0], ei[1]
    core_of = trg // cfg.shard

    # per-core edge lists (+ optional table compaction: shard rows first,
    # then the core's out-of-shard edge sources)
    core_esrc, core_trel, core_nodes = [], [], []
    for k in range(cfg.ncores):
        m = core_of == k
        esrc_k = src[m]
        trel_k = trg[m] - k * cfg.shard
        if cfg.compact:
            lo, hi = k * cfg.shard, (k + 1) * cfg.shard
            ext = np.unique(esrc_k[(esrc_k < lo) | (esrc_k >= hi)])
            nodes_k = np.concatenate([np.arange(lo, hi), ext])
            remap = np.empty(cfg.N, np.int64)
            remap[nodes_k] = np.arange(len(nodes_k))
            esrc_k = remap[esrc_k]
        else:
            nodes_k = None
        core_esrc.append(esrc_k)
        core_trel.append(trel_k)
        core_nodes.append(nodes_k)

    if cfg.compact:
        cfg.set_table_rows(max(len(n) for n in core_nodes))

    counts = np.zeros((cfg.ncores, cfg.NW, cfg.NB), np.int64)
    for k in range(cfg.ncores):
        counts[k] = np.bincount(
            (core_trel[k] // P) * cfg.NB + core_esrc[k] // cfg.bucket,
            minlength=cfg.NW * cfg.NB).reshape(cfg.NW, cfg.NB)
    sched = build_schedule(cfg, counts)

    wtp, wap = pack_w(cfg, W, a_src, a_tgt)
    av = np.full((P, 1), prelu_a, np.float32)
    if not cfg.compact:
        xt = pack_xt(cfg, X)

    in_maps = []
    for k in range(cfg.ncores):
        g1i_k, oh_k = prep_core(cfg, sched, core_esrc[k], core_trel[k])
        xt_k = pack_xt(cfg, X[core_nodes[k]]) if cfg.compact else xt
        in_maps.append({
            "xt": xt_k, "wt": wtp, "wa": wap,
            "g1i": g1i_k, "ohd": oh_k, "avec": av,
        })

    def assemble(core_outs):
        return np.concatenate(
            [np.asarray(o["out"][: cfg.shard], np.float32) for o in core_outs], axis=0)

    return sched, in_maps, assemble


_BUILT = {}


def _get_built(cfg: Cfg, sched: Schedule):
    key = (cfg.N, cfg.E, cfg.HID, cfg.HEADS, cfg.ncores, cfg.bucket,
           tuple(sched.TW), sched.idxcols)
    if key not in _BUILT:
        _BUILT[key] = build_nc(cfg, sched)
    return _BUILT[key]


def kernel(**inputs):
    from concourse.bass_utils import run_bass_kernel_spmd

    cfg = Cfg()
    sched, in_maps, assemble = prepare(cfg, inputs)
    nc = _get_built(cfg, sched)
    res = run_bass_kernel_spmd(nc, in_maps, core_ids=list(range(cfg.ncores)))
    return assemble(res.results)



# revision 33
# speedup vs baseline: 7.8857x; 1.2510x over previous
"""GAT layer (multi-head graph attention) on 8 TRN2 NeuronCores.

Strategy (per sharding hint): destination nodes are sharded across the 8
cores.  Each core:
  phase 0: computes s_tgt for its 12.5k shard targets (small GEMM against
           the a_tgt-folded weight) into a dedicated DRAM tensor.
  phase 1: computes the projection table for the ~63k unique edge-source
           nodes it needs (compacted, per-core order), packed as
           [proj bf16 | s_src f32] rows (1056B used of a 1280B pitch).
           The per-core row ORDER is chosen by an interval matching so
           that every destination window's sources fall inside ONE
           sliding 32768-row range -> a single dma_gather per window.
  phase 2: walks its shard's destination windows (128 targets / window).
           One dma_gather pulls the window's source rows (int16 indices
           against the window's compile-time base), scores -> leaky-relu
           -> exp (ScalarE-expanded to per-feature lanes) run batched
           per window, and fp8 one-hot matmuls (host-streamed)
           accumulate the softmax denominator and the weighted
           aggregation in PSUM.  Softmax division + PReLU at flush.

kernel(**inputs) takes the FULL inputs and returns the FULL output.
"""

import heapq
from dataclasses import dataclass

import numpy as np
import ml_dtypes

BF16 = ml_dtypes.bfloat16
P = 128
GR = 32768        # gather index range (int16)
XJ = 4            # projection tiles per xt DMA block


def _ceil(a, b):
    return -(-a // b)


@dataclass
class Cfg:
    N: int = 100000
    E: int = 800000
    HID: int = 512
    HEADS: int = 8
    ncores: int = 8
    leak: float = 0.01
    oh_bf16: bool = False  # one-hot stream dtype (fp8 halves DMA; bf16 fallback)
    out_bf16: bool = True  # device writes bf16 output (cast to f32 on host)

    def __post_init__(self):
        assert self.N % self.ncores == 0
        self.F = self.HID // self.HEADS
        self.shard = self.N // self.ncores
        self.NW = _ceil(self.shard, P)          # windows per core
        self.NT = _ceil(self.N, P)              # projection tiles (pre-compact)
        self.NPAD = self.NT * P
        self.KP = min(self.HID, P)              # contraction partitions
        self.KT = self.HID // self.KP           # contraction tiles
        row_bytes = self.HID * 2 + 2 * self.HEADS * 4
        self.row_bytes = _ceil(row_bytes, 256) * 256
        self.row_bf = self.row_bytes // 2
        self.row_f32 = self.row_bytes // 4
        self.s_src_off = self.HID // 2          # f32 col of s_src in a row
        self.row_w32 = self.s_src_off + self.HEADS  # f32 cols actually written

    def set_table_rows(self, nrows: int):
        """Size the projection table (compacted per-core source order)."""
        self.NT = _ceil(nrows, P)
        self.NPAD = self.NT * P


@dataclass
class Schedule:
    """Core-independent (uniform) phase-2 schedule: one gather per window."""
    bases: list        # per-window gather base row (compile-time, all cores)
    seg: list          # per-window slot count (128-mult)
    nreal: list        # per-window gather count (16-mult, max over cores)
    idxcol0: list      # per-window idx column offset
    TW: list           # tiles per window
    TWmax: int
    TT: int            # total tiles
    tile_base: list    # first global tile index of each window
    idxcols: int       # total int16 idx columns (16-wrap rows)


def build_schedule(cfg: Cfg, bases, counts) -> Schedule:
    """counts: [ncores, NW] per-window edge counts."""
    maxcnt = counts.max(axis=0)
    nreal, seg, TW, tile_base, idxcol0 = [], [], [], [], []
    idxcol = 0
    tt = 0
    for w in range(cfg.NW):
        nr = max(16, _ceil(int(maxcnt[w]), 16) * 16)
        sg = _ceil(nr, P) * P
        nreal.append(nr)
        seg.append(sg)
        TW.append(sg // P)
        tile_base.append(tt)
        idxcol0.append(idxcol)
        tt += sg // P
        idxcol += sg // 16
    return Schedule(bases=list(bases), seg=seg, nreal=nreal, idxcol0=idxcol0,
                    TW=TW, TWmax=max(TW), TT=tt, tile_base=tile_base,
                    idxcols=idxcol)


def match_positions(nfirst, nlast, bases, npad):
    """Assign each node a table position p with
    bases[last] <= p < bases[first] + GR (interval point matching).

    nfirst/nlast: per-node first/last window. Returns pos array."""
    lo = bases[nlast]                      # lower bound per node
    ub = bases[nfirst] + GR                # exclusive upper bound per node
    order = np.argsort(lo, kind="stable")
    pos = np.empty(len(lo), np.int64)
    heap = []
    oi = 0
    n = len(lo)
    done = 0
    for p in range(npad):
        while oi < n and lo[order[oi]] <= p:
            node = order[oi]
            heapq.heappush(heap, (int(ub[node]), int(node)))
            oi += 1
        if not heap:
            continue
        u, node = heapq.heappop(heap)
        assert u > p, f"interval matching infeasible at position {p}"
        pos[node] = p
        done += 1
        if done == n:
            break
    assert done == n, f"only placed {done}/{n} nodes"
    return pos


def prep_core(cfg: Cfg, sched: Schedule, epos, trel):
    """Per-core input arrays: g1 idx stream and one-hot stream.

    epos: per-edge source POSITION in this core's table.
    trel: per-edge target relative to the core's shard base.
    """
    oh_dt = BF16 if cfg.oh_bf16 else ml_dtypes.float8_e4m3
    win = trel // P
    # order edges by (window, position): position-ascending per window makes
    # the gather's HBM reads address-monotonic (row locality)
    order = np.lexsort((epos, win))
    epos, trel, win = (a[order] for a in (epos, trel, win))

    g1i = np.zeros((P, sched.idxcols), np.int16)
    oh = np.zeros((P, sched.TT, 2, P), oh_dt)

    starts = np.searchsorted(win, np.arange(cfg.NW), side="left")
    ends = np.searchsorted(win, np.arange(cfg.NW), side="right")

    for w in range(cfg.NW):
        lo, hi = int(starts[w]), int(ends[w])
        cnt = hi - lo
        nslots = sched.seg[w]
        assert cnt <= nslots
        rel = epos[lo:hi] - sched.bases[w]
        assert cnt == 0 or (rel.min() >= 0 and rel.max() < GR), \
            f"window {w}: idx out of range"
        idx = np.zeros(nslots, np.int16)
        idx[:cnt] = rel.astype(np.int16)
        if 0 < cnt < nslots:
            idx[cnt:] = idx[cnt - 1]   # pads re-hit the open HBM row
        blk = idx.reshape(nslots // 16, 16).T          # [16, cols]
        c0 = sched.idxcol0[w]
        g1i[:, c0:c0 + nslots // 16] = np.tile(blk, (8, 1))
        # one-hots for this window's tiles
        tloc = (trel[lo:hi] - w * P).astype(np.int64)  # [cnt] in [0,128)
        t0 = sched.tile_base[w]
        for j in range(nslots // P):
            s0, s1 = j * P, min((j + 1) * P, cnt)
            if s1 <= s0:
                continue
            rows = np.arange(s0, s1) - s0
            cols = tloc[s0:s1]
            oh[rows, t0 + j, 0, cols] = oh_dt(1.0)
            oh[cols, t0 + j, 1, rows] = oh_dt(1.0)
    return g1i, oh


def pack_xt(cfg: Cfg, X: np.ndarray, nt: int) -> np.ndarray:
    """X [rows, HID] f32 -> bf16 packed [KP, NJ, KT, XJ*P]:
    (p, jb, ki, j*P+n) = X[(jb*XJ+j)*P+n, ki*KP+p]."""
    nj = _ceil(nt, XJ)
    Xp = np.zeros((nj * XJ * P, cfg.HID), np.float32)
    Xp[: X.shape[0]] = np.asarray(X, np.float32)
    Xb = Xp.astype(BF16)
    v = Xb.reshape(nj, XJ * P, cfg.KT, cfg.KP)
    return np.ascontiguousarray(v.transpose(3, 0, 2, 1))


def pack_w(cfg: Cfg, W, a_src, a_tgt):
    """Returns wt [KP, KT, HID] bf16 and wa [KP, KT, 2*HEADS] bf16."""
    WT = W.T.astype(np.float32)                       # [HID(d), HID(o)]
    wa_s = (W.reshape(cfg.HEADS, cfg.F, cfg.HID)
            * np.asarray(a_src, np.float32).reshape(cfg.HEADS, cfg.F, 1)).sum(1)
    wa_t = (W.reshape(cfg.HEADS, cfg.F, cfg.HID)
            * np.asarray(a_tgt, np.float32).reshape(cfg.HEADS, cfg.F, 1)).sum(1)
    WA = np.concatenate([wa_s.T, wa_t.T], axis=1)     # [d, 2H]
    wt = np.ascontiguousarray(
        WT.astype(BF16).reshape(cfg.KT, cfg.KP, cfg.HID).transpose(1, 0, 2))
    wa = np.ascontiguousarray(
        WA.astype(BF16).reshape(cfg.KT, cfg.KP, 2 * cfg.HEADS).transpose(1, 0, 2))
    return wt, wa


def _bcast_last(ap, n):
    """Append a 0-stride broadcast dim of size n to an AP."""
    import concourse.bass as bass
    lst = [list(x) for x in ap.ap] + [[0, n]]
    return bass.AP(ap.tensor, ap.offset, lst)


def build_nc(cfg: Cfg, sched: Schedule):
    import concourse.bacc as bacc
    import concourse.mybir as mybir
    from concourse.tile import TileContext

    dt = mybir.dt
    oh_mdt = dt.bfloat16 if cfg.oh_bf16 else dt.float8e4
    out_mdt = dt.bfloat16 if cfg.out_bf16 else dt.float32
    H, HID, KT, KP = cfg.HEADS, cfg.HID, cfg.KT, cfg.KP

    nc = bacc.Bacc("TRN2", target_bir_lowering=False)

    NJ = _ceil(cfg.NT, XJ)
    NJ2 = _ceil(cfg.NW, XJ)
    xt = nc.dram_tensor("xt", [KP, NJ, KT, XJ * P], dt.bfloat16,
                        kind="ExternalInput")
    xs = nc.dram_tensor("xs", [KP, NJ2, KT, XJ * P], dt.bfloat16,
                        kind="ExternalInput")
    wt = nc.dram_tensor("wt", [KP, KT, HID], dt.bfloat16, kind="ExternalInput")
    wa = nc.dram_tensor("wa", [KP, KT, 2 * H], dt.bfloat16, kind="ExternalInput")
    g1i = nc.dram_tensor("g1i", [P, sched.idxcols], dt.int16, kind="ExternalInput")
    ohd = nc.dram_tensor("ohd", [P, sched.TT, 2, P], oh_mdt, kind="ExternalInput")
    avec = nc.dram_tensor("avec", [P, 1], dt.float32, kind="ExternalInput")
    out = nc.dram_tensor("out", [cfg.NW * P, HID], out_mdt, kind="ExternalOutput")

    with TileContext(nc) as tc:
        with tc.tile_pool(name="const", bufs=1) as cpool, \
             tc.tile_pool(name="dram", bufs=1, space="DRAM") as dpool:
            table = dpool.tile([cfg.NPAD, cfg.row_bf], dt.bfloat16)
            stgt_d = dpool.tile([NJ2 * XJ * P, H], dt.float32)
            wt_sb = cpool.tile([KP, KT, HID], dt.bfloat16)
            nc.sync.dma_start(out=wt_sb[:], in_=wt[:, :, :])
            wa_sb = cpool.tile([KP, KT, 2 * H], dt.bfloat16)
            nc.sync.dma_start(out=wa_sb[:], in_=wa[:, :, :])
            a_sb = cpool.tile([P, 1], dt.float32)
            nc.sync.dma_start(out=a_sb[:], in_=avec[:, :])
            g1i_sb = cpool.tile([P, sched.idxcols], dt.int16)
            nc.sync.dma_start(out=g1i_sb[:], in_=g1i[:, :])

            # ---------- phase 0: s_tgt for the local shard ----------
            with tc.tile_pool(name="p0", bufs=2) as x2pool, \
                 tc.tile_pool(name="p0ps", bufs=2, space="PSUM") as psp0, \
                 tc.tile_pool(name="p0st", bufs=2) as st0pool:
                for jb in range(NJ2):
                    xtile = x2pool.tile([KP, KT, XJ * P], dt.bfloat16, tag="x2")
                    nc.sync.dma_start(out=xtile[:], in_=xs[:, jb, :, :])
                    stg3 = st0pool.tile([P, XJ, H], dt.float32, tag="stg3")
                    for j in range(XJ):
                        ps3 = psp0.tile([P, H], dt.float32, tag="ps3")
                        for ki in range(KT):
                            nc.tensor.matmul(ps3[:],
                                             xtile[:, ki, j * P:(j + 1) * P],
                                             wa_sb[:, ki, H:2 * H],
                                             start=(ki == 0), stop=(ki == KT - 1))
                        nc.scalar.copy(out=stg3[:, j, :], in_=ps3[:])
                    dst = stgt_d[jb * XJ * P:(jb + 1) * XJ * P, :].rearrange(
                        "(j p) h -> p j h", p=P)
                    nc.sync.dma_start(out=dst, in_=stg3[:])

            # ---------- phase 1: projection table (sources) ----------
            # rows are written 1056B-dense (proj + s_src); the DRAM row
            # tail (1056:1280) stays uninitialized and is never read.
            with tc.tile_pool(name="p1", bufs=2) as xpool, \
                 tc.tile_pool(name="p1ps", bufs=3, space="PSUM") as psp1, \
                 tc.tile_pool(name="p1ps2", bufs=2, space="PSUM") as psp2, \
                 tc.tile_pool(name="p1st", bufs=3) as stpool:
                SJ = 2  # staging tiles per table write
                for jb in range(NJ):
                    xtile = xpool.tile([KP, KT, XJ * P], dt.bfloat16, tag="x")
                    nc.sync.dma_start(out=xtile[:], in_=xt[:, jb, :, :])
                    for j in range(XJ):
                        ps1 = psp1.tile([P, HID], dt.float32, tag="ps1")
                        ps2 = psp2.tile([P, H], dt.float32, tag="ps2")
                        for ki in range(KT):
                            nc.tensor.matmul(ps1[:], xtile[:, ki, j * P:(j + 1) * P],
                                             wt_sb[:, ki, :],
                                             start=(ki == 0), stop=(ki == KT - 1))
                            nc.tensor.matmul(ps2[:], xtile[:, ki, j * P:(j + 1) * P],
                                             wa_sb[:, ki, 0:H],
                                             start=(ki == 0), stop=(ki == KT - 1))
                        jj = jb * XJ + j
                        if jj % SJ == 0:
                            stg = stpool.tile([P, SJ, cfg.row_bf], dt.bfloat16,
                                              tag="stg")
                            stg32 = stg.bitcast(dt.float32)
                        sl = jj % SJ
                        nc.scalar.copy(out=stg[:, sl, 0:HID], in_=ps1[:])
                        nc.scalar.copy(
                            out=stg32[:, sl, cfg.s_src_off:cfg.s_src_off + H],
                            in_=ps2[:])
                        if sl == SJ - 1:
                            r0 = (jj - sl) * P
                            dst = table[r0:r0 + SJ * P, 0:2 * cfg.row_w32].rearrange(
                                "(j p) c -> p j c", p=P)
                            nc.sync.dma_start(out=dst,
                                              in_=stg[:, :, 0:2 * cfg.row_w32])

            tc.strict_bb_all_engine_barrier()

            # ---------- phase 1.5: resident s_tgt (hi/lo bf16) ----------
            s_ap = stgt_d[0:cfg.NW * P, :].rearrange("(w p) h -> p w h", p=P)
            s_all = cpool.tile([P, cfg.NW, H], dt.float32)
            nc.sync.dma_start(out=s_all[:], in_=s_ap)
            s_hilo = cpool.tile([P, cfg.NW, 2, H], dt.bfloat16)
            s_hi32 = cpool.tile([P, cfg.NW, H], dt.float32)
            nc.vector.tensor_copy(out=s_hilo[:, :, 0, :], in_=s_all[:])
            nc.vector.tensor_copy(out=s_hi32[:], in_=s_hilo[:, :, 0, :])
            nc.vector.tensor_tensor(out=s_hilo[:, :, 1, :], in0=s_all[:],
                                    in1=s_hi32[:], op=mybir.AluOpType.subtract)

            # ---------- phase 2: windows ----------
            G1B = 3
            with tc.tile_pool(name="p2", bufs=2) as pool, \
                 tc.tile_pool(name="p2g", bufs=G1B) as gpool, \
                 tc.tile_pool(name="p2ps", bufs=2, space="PSUM") as pps:
                # one-time memset of the gather slots: trailing (pad) slots
                # keep stale SBUF, and garbage bits could be NaN/Inf and
                # poison 0*NaN in the aggregation matmuls.
                for _ in range(G1B):
                    g1t = gpool.tile([P, sched.TWmax, cfg.row_bf], dt.bfloat16,
                                     tag="g1t")
                    nc.vector.memset(g1t.bitcast(dt.float32)[:], 0.0)
                for w in range(cfg.NW):
                    Tw = sched.TW[w]
                    nreal = sched.nreal[w]
                    base = sched.bases[w]
                    rows = min(GR, cfg.NPAD - base)
                    g1t = gpool.tile([P, sched.TWmax, cfg.row_bf], dt.bfloat16,
                                     tag="g1t")
                    # split into <=896-idx calls: 56 descriptors per SDMA
                    # engine keeps each gather inside the 64-desc packet limit
                    GCH = 896
                    done = 0
                    while done < nreal:
                        n_c = min(GCH, nreal - done)
                        c0 = sched.idxcol0[w] + done // 16
                        nc.gpsimd.dma_gather(
                            g1t[:, done // P:done // P + _ceil(n_c, P), :],
                            table[base:base + rows, :],
                            g1i_sb[:, c0:c0 + _ceil(n_c, 16)],
                            n_c, n_c, cfg.row_bf)
                        done += n_c
                    jb = sched.tile_base[w]
                    oht = pool.tile([P, sched.TWmax, 2, P], oh_mdt, tag="oht")
                    nc.sync.dma_start(out=oht[:, :Tw, :, :],
                                      in_=ohd[:, jb:jb + Tw, :, :])

                    # s_tgt expansion (per tile) via transposed one-hot matmul
                    stgt = pps.tile([P, sched.TWmax, 2, H], dt.float32, tag="stgt")
                    for t in range(Tw):
                        nc.tensor.matmul(stgt[:, t, :, :], oht[:, t, 1, :],
                                         s_hilo[:, w, :, :], start=True, stop=True)
                    g1t32 = g1t.bitcast(dt.float32)
                    s_sum = pool.tile([P, sched.TWmax, H], dt.float32, tag="s_sum")
                    s_act = pool.tile([P, sched.TWmax, H], dt.float32, tag="s_act")
                    nc.vector.tensor_tensor(
                        out=s_sum[:, :Tw, :], in0=stgt[:, :Tw, 0, :],
                        in1=g1t32[:, :Tw, cfg.s_src_off:cfg.s_src_off + H],
                        op=mybir.AluOpType.add)
                    nc.vector.tensor_tensor(
                        out=s_act[:, :Tw, :], in0=stgt[:, :Tw, 1, :],
                        in1=s_sum[:, :Tw, :], op=mybir.AluOpType.add)
                    nc.vector.scalar_tensor_tensor(
                        out=s_sum[:, :Tw, :], in0=s_act[:, :Tw, :], scalar=cfg.leak,
                        in1=s_act[:, :Tw, :], op0=mybir.AluOpType.mult,
                        op1=mybir.AluOpType.max)
                    # exp, expanded to F copies per head on ScalarE (0-stride
                    # read) so the big multiply below runs in DVE 2x mode
                    exp_r = pool.tile([P, sched.TWmax, HID], dt.bfloat16, tag="exp_r")
                    exp_in = _bcast_last(s_sum[:, :Tw, :], cfg.F)
                    nc.scalar.activation(out=exp_r[:, :Tw, :].rearrange(
                                             "p t (h f) -> p t h f", h=H),
                                         in_=exp_in,
                                         func=mybir.ActivationFunctionType.Exp)

                    w_t = pool.tile([P, sched.TWmax, HID], dt.bfloat16, tag="w_t")
                    nc.vector.tensor_tensor(out=w_t[:, :Tw, :],
                                            in0=g1t[:, :Tw, 0:HID],
                                            in1=exp_r[:, :Tw, :],
                                            op=mybir.AluOpType.mult)

                    exp_h = exp_r[:].rearrange("p t (h f) -> p t h f", h=H)
                    agg = pps.tile([P, HID], dt.float32, tag="agg")
                    den = pps.tile([P, H], dt.float32, tag="den")
                    for t in range(Tw):
                        nc.tensor.matmul(agg[:], oht[:, t, 0, :], w_t[:, t, :],
                                         start=(t == 0), stop=(t == Tw - 1))
                        nc.tensor.matmul(den[:], oht[:, t, 0, :], exp_h[:, t, :, 0],
                                         start=(t == 0), stop=(t == Tw - 1))

                    # flush: softmax divide + PReLU
                    den_sb = pool.tile([P, H, 1], dt.float32, tag="den_sb")
                    recip = pool.tile([P, H, 1], dt.float32, tag="recip")
                    nc.vector.tensor_scalar_add(out=den_sb[:, :, 0], in0=den[:],
                                                scalar1=1e-16)
                    nc.vector.reciprocal(out=recip[:], in_=den_sb[:])
                    z = pool.tile([P, HID], dt.float32, tag="z")
                    agg4 = agg[:].rearrange("p (h f) -> p h f", h=H)
                    z4 = z[:].rearrange("p (h f) -> p h f", h=H)
                    nc.vector.tensor_tensor(out=z4, in0=agg4,
                                            in1=_bcast_last(recip[:, :, 0], cfg.F),
                                            op=mybir.AluOpType.mult)
                    res = pool.tile([P, HID], out_mdt, tag="res")
                    nc.vector.scalar_tensor_tensor(
                        out=res[:], in0=z[:], scalar=a_sb[:, 0:1], in1=z[:],
                        op0=mybir.AluOpType.mult, op1=mybir.AluOpType.max)
                    nc.sync.dma_start(out=out[w * P:(w + 1) * P, :], in_=res[:])

    nc.compile()
    return nc


def prepare(cfg: Cfg, inputs):
    """Host-side prep. Returns (sched, in_maps, assemble)."""
    X = np.asarray(inputs["in_nodes_features"], np.float32)
    ei = np.asarray(inputs["edge_index"], np.int64)
    W = np.asarray(inputs["W"], np.float32)
    b_lin = np.asarray(inputs["b_lin"], np.float32)
    a_src = np.asarray(inputs["a_src"], np.float32)
    a_tgt = np.asarray(inputs["a_tgt"], np.float32)
    bias = np.asarray(inputs["bias"], np.float32)
    prelu_a = float(np.asarray(inputs["prelu_a"], np.float32))

    assert np.all(b_lin == 0) and np.all(bias == 0), "nonzero bias unsupported"
    assert 0.0 <= prelu_a <= 1.0, "prelu_a outside [0,1] unsupported"

    src, trg = ei[0], ei[1]
    core_of = trg // cfg.shard

    # per-core edge lists + per-node first/last windows
    core_esrc, core_trel, core_uniq, core_fw, core_lw = [], [], [], [], []
    for k in range(cfg.ncores):
        m = core_of == k
        esrc_k = src[m]
        trel_k = trg[m] - k * cfg.shard
        win_k = trel_k // P
        first = np.full(cfg.N, cfg.NW, np.int64)
        last = np.full(cfg.N, -1, np.int64)
        np.minimum.at(first, esrc_k, win_k)
        np.maximum.at(last, esrc_k, win_k)
        uniq = np.nonzero(last >= 0)[0]
        core_esrc.append(esrc_k)
        core_trel.append(trel_k)
        core_uniq.append(uniq)
        core_fw.append(first[uniq])
        core_lw.append(last[uniq])

    ntc_max = max(len(u) for u in core_uniq)
    cfg.set_table_rows(ntc_max)
    assert cfg.NPAD - GR < GR, "base span must fit the int16 gather range"

    # compile-time window bases: Hall-feasible midpoints, monotone
    lo = np.zeros(cfg.NW, np.int64)
    hi = np.full(cfg.NW, max(0, cfg.NPAD - GR), np.int64)
    for k in range(cfg.ncores):
        U = np.cumsum(np.bincount(core_fw[k], minlength=cfg.NW))   # first<=w
        CL = np.concatenate([[0], np.cumsum(
            np.bincount(core_lw[k], minlength=cfg.NW))[:-1]])      # last<w
        lo = np.maximum(lo, U - GR)
        hi = np.minimum(hi, CL)
    lo = np.maximum(lo, 0)
    assert np.all(lo <= hi), "no feasible window bases"
    bases = np.maximum.accumulate((lo + hi) // 2)

    counts = np.zeros((cfg.ncores, cfg.NW), np.int64)
    core_pos = []
    for k in range(cfg.ncores):
        pos = match_positions(core_fw[k], core_lw[k], bases, cfg.NPAD)
        core_pos.append(pos)
        counts[k] = np.bincount(core_trel[k] // P, minlength=cfg.NW)

    sched = build_schedule(cfg, bases, counts)

    wtp, wap = pack_w(cfg, W, a_src, a_tgt)
    av = np.full((P, 1), prelu_a, np.float32)

    in_maps = []
    for k in range(cfg.ncores):
        remap = np.empty(cfg.N, np.int64)
        remap[core_uniq[k]] = core_pos[k]
        epos_k = remap[core_esrc[k]]
        g1i_k, oh_k = prep_core(cfg, sched, epos_k, core_trel[k])
        Xg = np.zeros((cfg.NPAD, cfg.HID), np.float32)
        Xg[core_pos[k]] = X[core_uniq[k]]
        xt_k = pack_xt(cfg, Xg, cfg.NT)
        xs_k = pack_xt(cfg, X[k * cfg.shard:(k + 1) * cfg.shard], cfg.NW)
        in_maps.append({
            "xt": xt_k, "xs": xs_k, "wt": wtp, "wa": wap,
            "g1i": g1i_k, "ohd": oh_k, "avec": av,
        })

    def assemble(core_outs):
        return np.concatenate(
            [np.asarray(o["out"][: cfg.shard], np.float32) for o in core_outs],
            axis=0)

    return sched, in_maps, assemble


_BUILT = {}


def _get_built(cfg: Cfg, sched: Schedule):
    key = (cfg.N, cfg.E, cfg.HID, cfg.HEADS, cfg.ncores, cfg.NT,
           tuple(sched.TW), tuple(sched.bases), sched.idxcols)
    if key not in _BUILT:
        _BUILT[key] = build_nc(cfg, sched)
    return _BUILT[key]


def kernel(**inputs):
    from concourse.bass_utils import run_bass_kernel_spmd

    cfg = Cfg()
    sched, in_maps, assemble = prepare(cfg, inputs)
    nc = _get_built(cfg, sched)
    res = run_bass_kernel_spmd(nc, in_maps, core_ids=list(range(cfg.ncores)))
    return assemble(res.results)


# revision 34
# speedup vs baseline: 8.0004x; 1.0145x over previous
"""GAT layer (multi-head graph attention) on 8 TRN2 NeuronCores.

Strategy (per sharding hint): destination nodes are sharded across the 8
cores.  Each core:
  phase 0: computes s_tgt for its 12.5k shard targets (small GEMM against
           the a_tgt-folded weight) into a dedicated DRAM tensor.
  phase 1: computes the projection table for the ~63k unique edge-source
           nodes it needs (compacted, per-core order), packed as
           [proj bf16 | s_src f32] rows (1056B used of a 1280B pitch).
           The per-core row ORDER is chosen by an interval matching so
           that every destination window's sources fall inside ONE
           sliding 32768-row range -> a single dma_gather per window.
  phase 2: walks its shard's destination windows (128 targets / window).
           One dma_gather pulls the window's source rows (int16 indices
           against the window's compile-time base), scores -> leaky-relu
           -> exp (ScalarE-expanded to per-feature lanes) run batched
           per window, and fp8 one-hot matmuls (host-streamed)
           accumulate the softmax denominator and the weighted
           aggregation in PSUM.  Softmax division + PReLU at flush.

kernel(**inputs) takes the FULL inputs and returns the FULL output.
"""

import heapq
from dataclasses import dataclass

import numpy as np
import ml_dtypes

BF16 = ml_dtypes.bfloat16
P = 128
GR = 32768        # gather index range (int16)
XJ = 4            # projection tiles per xt DMA block


def _ceil(a, b):
    return -(-a // b)


@dataclass
class Cfg:
    N: int = 100000
    E: int = 800000
    HID: int = 512
    HEADS: int = 8
    ncores: int = 8
    leak: float = 0.01
    oh_bf16: bool = False  # one-hot stream dtype (fp8 halves DMA; bf16 fallback)
    out_bf16: bool = True  # device writes bf16 output (cast to f32 on host)

    def __post_init__(self):
        assert self.N % self.ncores == 0
        self.F = self.HID // self.HEADS
        self.shard = self.N // self.ncores
        self.NW = _ceil(self.shard, P)          # windows per core
        self.NT = _ceil(self.N, P)              # projection tiles (pre-compact)
        self.NPAD = self.NT * P
        self.KP = min(self.HID, P)              # contraction partitions
        self.KT = self.HID // self.KP           # contraction tiles
        row_bytes = self.HID * 2 + 2 * self.HEADS * 4
        self.row_bytes = _ceil(row_bytes, 256) * 256
        self.row_bf = self.row_bytes // 2
        self.row_f32 = self.row_bytes // 4
        self.s_src_off = self.HID // 2          # f32 col of s_src in a row
        self.row_w32 = self.s_src_off + self.HEADS  # f32 cols actually written

    def set_table_rows(self, nrows: int):
        """Size the projection table (compacted per-core source order)."""
        self.NT = _ceil(nrows, P)
        self.NPAD = self.NT * P


@dataclass
class Schedule:
    """Core-independent (uniform) phase-2 schedule: one gather per window."""
    bases: list        # per-window gather base row (compile-time, all cores)
    seg: list          # per-window slot count (128-mult)
    nreal: list        # per-window gather count (16-mult, max over cores)
    idxcol0: list      # per-window idx column offset
    TW: list           # tiles per window
    TWmax: int
    TT: int            # total tiles
    tile_base: list    # first global tile index of each window
    idxcols: int       # total int16 idx columns (16-wrap rows)


def build_schedule(cfg: Cfg, bases, counts) -> Schedule:
    """counts: [ncores, NW] per-window edge counts."""
    maxcnt = counts.max(axis=0)
    nreal, seg, TW, tile_base, idxcol0 = [], [], [], [], []
    idxcol = 0
    tt = 0
    for w in range(cfg.NW):
        nr = max(16, _ceil(int(maxcnt[w]), 16) * 16)
        sg = _ceil(nr, P) * P
        nreal.append(nr)
        seg.append(sg)
        TW.append(sg // P)
        tile_base.append(tt)
        idxcol0.append(idxcol)
        tt += sg // P
        idxcol += sg // 16
    return Schedule(bases=list(bases), seg=seg, nreal=nreal, idxcol0=idxcol0,
                    TW=TW, TWmax=max(TW), TT=tt, tile_base=tile_base,
                    idxcols=idxcol)


def match_positions(nfirst, nlast, bases, npad):
    """Assign each node a table position p with
    bases[last] <= p < bases[first] + GR (interval point matching).

    nfirst/nlast: per-node first/last window. Returns pos array."""
    lo = bases[nlast]                      # lower bound per node
    ub = bases[nfirst] + GR                # exclusive upper bound per node
    order = np.argsort(lo, kind="stable")
    pos = np.empty(len(lo), np.int64)
    heap = []
    oi = 0
    n = len(lo)
    done = 0
    for p in range(npad):
        while oi < n and lo[order[oi]] <= p:
            node = order[oi]
            heapq.heappush(heap, (int(ub[node]), int(node)))
            oi += 1
        if not heap:
            continue
        u, node = heapq.heappop(heap)
        assert u > p, f"interval matching infeasible at position {p}"
        pos[node] = p
        done += 1
        if done == n:
            break
    assert done == n, f"only placed {done}/{n} nodes"
    return pos


def prep_core(cfg: Cfg, sched: Schedule, epos, trel):
    """Per-core input arrays: g1 idx stream and one-hot stream.

    epos: per-edge source POSITION in this core's table.
    trel: per-edge target relative to the core's shard base.
    """
    oh_dt = BF16 if cfg.oh_bf16 else ml_dtypes.float8_e4m3
    win = trel // P
    # order edges by (window, position): position-ascending per window makes
    # the gather's HBM reads address-monotonic (row locality)
    order = np.lexsort((epos, win))
    epos, trel, win = (a[order] for a in (epos, trel, win))

    g1i = np.zeros((P, sched.idxcols), np.int16)
    oh = np.zeros((P, sched.TT, 2, P), oh_dt)

    starts = np.searchsorted(win, np.arange(cfg.NW), side="left")
    ends = np.searchsorted(win, np.arange(cfg.NW), side="right")

    for w in range(cfg.NW):
        lo, hi = int(starts[w]), int(ends[w])
        cnt = hi - lo
        nslots = sched.seg[w]
        assert cnt <= nslots
        rel = epos[lo:hi] - sched.bases[w]
        assert cnt == 0 or (rel.min() >= 0 and rel.max() < GR), \
            f"window {w}: idx out of range"
        idx = np.zeros(nslots, np.int16)
        idx[:cnt] = rel.astype(np.int16)
        if 0 < cnt < nslots:
            idx[cnt:] = idx[cnt - 1]   # pads re-hit the open HBM row
        blk = idx.reshape(nslots // 16, 16).T          # [16, cols]
        c0 = sched.idxcol0[w]
        g1i[:, c0:c0 + nslots // 16] = np.tile(blk, (8, 1))
        # one-hots for this window's tiles
        tloc = (trel[lo:hi] - w * P).astype(np.int64)  # [cnt] in [0,128)
        t0 = sched.tile_base[w]
        for j in range(nslots // P):
            s0, s1 = j * P, min((j + 1) * P, cnt)
            if s1 <= s0:
                continue
            rows = np.arange(s0, s1) - s0
            cols = tloc[s0:s1]
            oh[rows, t0 + j, 0, cols] = oh_dt(1.0)
            oh[cols, t0 + j, 1, rows] = oh_dt(1.0)
    return g1i, oh


def pack_xt(cfg: Cfg, X: np.ndarray, nt: int) -> np.ndarray:
    """X [rows, HID] f32 -> bf16 packed [KP, NJ, KT, XJ*P]:
    (p, jb, ki, j*P+n) = X[(jb*XJ+j)*P+n, ki*KP+p]."""
    nj = _ceil(nt, XJ)
    Xp = np.zeros((nj * XJ * P, cfg.HID), np.float32)
    Xp[: X.shape[0]] = np.asarray(X, np.float32)
    Xb = Xp.astype(BF16)
    v = Xb.reshape(nj, XJ * P, cfg.KT, cfg.KP)
    return np.ascontiguousarray(v.transpose(3, 0, 2, 1))


def pack_w(cfg: Cfg, W, a_src, a_tgt):
    """Returns wt [KP, KT, HID] bf16 and wa [KP, KT, 2*HEADS] bf16."""
    WT = W.T.astype(np.float32)                       # [HID(d), HID(o)]
    wa_s = (W.reshape(cfg.HEADS, cfg.F, cfg.HID)
            * np.asarray(a_src, np.float32).reshape(cfg.HEADS, cfg.F, 1)).sum(1)
    wa_t = (W.reshape(cfg.HEADS, cfg.F, cfg.HID)
            * np.asarray(a_tgt, np.float32).reshape(cfg.HEADS, cfg.F, 1)).sum(1)
    WA = np.concatenate([wa_s.T, wa_t.T], axis=1)     # [d, 2H]
    wt = np.ascontiguousarray(
        WT.astype(BF16).reshape(cfg.KT, cfg.KP, cfg.HID).transpose(1, 0, 2))
    wa = np.ascontiguousarray(
        WA.astype(BF16).reshape(cfg.KT, cfg.KP, 2 * cfg.HEADS).transpose(1, 0, 2))
    return wt, wa


def _bcast_last(ap, n):
    """Append a 0-stride broadcast dim of size n to an AP."""
    import concourse.bass as bass
    lst = [list(x) for x in ap.ap] + [[0, n]]
    return bass.AP(ap.tensor, ap.offset, lst)


def build_nc(cfg: Cfg, sched: Schedule):
    import concourse.bacc as bacc
    import concourse.mybir as mybir
    from concourse.tile import TileContext

    dt = mybir.dt
    oh_mdt = dt.bfloat16 if cfg.oh_bf16 else dt.float8e4
    out_mdt = dt.bfloat16 if cfg.out_bf16 else dt.float32
    H, HID, KT, KP = cfg.HEADS, cfg.HID, cfg.KT, cfg.KP

    nc = bacc.Bacc("TRN2", target_bir_lowering=False)

    NJ = _ceil(cfg.NT, XJ)
    NJ2 = _ceil(cfg.NW, XJ)
    xt = nc.dram_tensor("xt", [KP, NJ, KT, XJ * P], dt.bfloat16,
                        kind="ExternalInput")
    xs = nc.dram_tensor("xs", [KP, NJ2, KT, XJ * P], dt.bfloat16,
                        kind="ExternalInput")
    wt = nc.dram_tensor("wt", [KP, KT, HID], dt.bfloat16, kind="ExternalInput")
    wa = nc.dram_tensor("wa", [KP, KT, 2 * H], dt.bfloat16, kind="ExternalInput")
    g1i = nc.dram_tensor("g1i", [P, sched.idxcols], dt.int16, kind="ExternalInput")
    ohd = nc.dram_tensor("ohd", [P, sched.TT, 2, P], oh_mdt, kind="ExternalInput")
    avec = nc.dram_tensor("avec", [P, 1], dt.float32, kind="ExternalInput")
    out = nc.dram_tensor("out", [cfg.NW * P, HID], out_mdt, kind="ExternalOutput")

    with TileContext(nc) as tc:
        with tc.tile_pool(name="const", bufs=1) as cpool, \
             tc.tile_pool(name="dram", bufs=1, space="DRAM") as dpool:
            table = dpool.tile([cfg.NPAD, cfg.row_bf], dt.bfloat16)
            stgt_d = dpool.tile([NJ2 * XJ * P, H], dt.float32)
            wt_sb = cpool.tile([KP, KT, HID], dt.bfloat16)
            nc.sync.dma_start(out=wt_sb[:], in_=wt[:, :, :])
            wa_sb = cpool.tile([KP, KT, 2 * H], dt.bfloat16)
            nc.sync.dma_start(out=wa_sb[:], in_=wa[:, :, :])
            a_sb = cpool.tile([P, 1], dt.float32)
            nc.sync.dma_start(out=a_sb[:], in_=avec[:, :])
            g1i_sb = cpool.tile([P, sched.idxcols], dt.int16)
            nc.sync.dma_start(out=g1i_sb[:], in_=g1i[:, :])

            # ---------- phase 0: s_tgt for the local shard ----------
            with tc.tile_pool(name="p0", bufs=2) as x2pool, \
                 tc.tile_pool(name="p0ps", bufs=2, space="PSUM") as psp0, \
                 tc.tile_pool(name="p0st", bufs=2) as st0pool:
                for jb in range(NJ2):
                    xtile = x2pool.tile([KP, KT, XJ * P], dt.bfloat16, tag="x2")
                    nc.sync.dma_start(out=xtile[:], in_=xs[:, jb, :, :])
                    stg3 = st0pool.tile([P, XJ, H], dt.float32, tag="stg3")
                    for j in range(XJ):
                        ps3 = psp0.tile([P, H], dt.float32, tag="ps3")
                        for ki in range(KT):
                            nc.tensor.matmul(ps3[:],
                                             xtile[:, ki, j * P:(j + 1) * P],
                                             wa_sb[:, ki, H:2 * H],
                                             start=(ki == 0), stop=(ki == KT - 1))
                        nc.scalar.copy(out=stg3[:, j, :], in_=ps3[:])
                    dst = stgt_d[jb * XJ * P:(jb + 1) * XJ * P, :].rearrange(
                        "(j p) h -> p j h", p=P)
                    nc.sync.dma_start(out=dst, in_=stg3[:])

            # ---------- phase 1: projection table (sources) ----------
            # rows are written 1056B-dense (proj + s_src); the DRAM row
            # tail (1056:1280) stays uninitialized and is never read.
            with tc.tile_pool(name="p1", bufs=2) as xpool, \
                 tc.tile_pool(name="p1ps", bufs=3, space="PSUM") as psp1, \
                 tc.tile_pool(name="p1ps2", bufs=2, space="PSUM") as psp2, \
                 tc.tile_pool(name="p1st", bufs=3) as stpool:
                SJ = 2  # staging tiles per table write
                for jb in range(NJ):
                    xtile = xpool.tile([KP, KT, XJ * P], dt.bfloat16, tag="x")
                    nc.sync.dma_start(out=xtile[:], in_=xt[:, jb, :, :])
                    for j in range(XJ):
                        ps1 = psp1.tile([P, HID], dt.float32, tag="ps1")
                        ps2 = psp2.tile([P, H], dt.float32, tag="ps2")
                        for ki in range(KT):
                            nc.tensor.matmul(ps1[:], xtile[:, ki, j * P:(j + 1) * P],
                                             wt_sb[:, ki, :],
                                             start=(ki == 0), stop=(ki == KT - 1))
                            nc.tensor.matmul(ps2[:], xtile[:, ki, j * P:(j + 1) * P],
                                             wa_sb[:, ki, 0:H],
                                             start=(ki == 0), stop=(ki == KT - 1))
                        jj = jb * XJ + j
                        if jj % SJ == 0:
                            stg = stpool.tile([P, SJ, cfg.row_bf], dt.bfloat16,
                                              tag="stg")
                            stg32 = stg.bitcast(dt.float32)
                        sl = jj % SJ
                        nc.scalar.copy(out=stg[:, sl, 0:HID], in_=ps1[:])
                        nc.scalar.copy(
                            out=stg32[:, sl, cfg.s_src_off:cfg.s_src_off + H],
                            in_=ps2[:])
                        if sl == SJ - 1:
                            r0 = (jj - sl) * P
                            dst = table[r0:r0 + SJ * P, 0:2 * cfg.row_w32].rearrange(
                                "(j p) c -> p j c", p=P)
                            nc.sync.dma_start(out=dst,
                                              in_=stg[:, :, 0:2 * cfg.row_w32])

            tc.strict_bb_all_engine_barrier()

            # ---------- phase 1.5: resident s_tgt (hi/lo bf16) ----------
            s_ap = stgt_d[0:cfg.NW * P, :].rearrange("(w p) h -> p w h", p=P)
            s_all = cpool.tile([P, cfg.NW, H], dt.float32)
            nc.sync.dma_start(out=s_all[:], in_=s_ap)
            s_hilo = cpool.tile([P, cfg.NW, 2, H], dt.bfloat16)
            s_hi32 = cpool.tile([P, cfg.NW, H], dt.float32)
            nc.vector.tensor_copy(out=s_hilo[:, :, 0, :], in_=s_all[:])
            nc.vector.tensor_copy(out=s_hi32[:], in_=s_hilo[:, :, 0, :])
            nc.vector.tensor_tensor(out=s_hilo[:, :, 1, :], in0=s_all[:],
                                    in1=s_hi32[:], op=mybir.AluOpType.subtract)

            # ---------- phase 2: windows ----------
            G1B = 3
            with tc.tile_pool(name="p2", bufs=2) as pool, \
                 tc.tile_pool(name="p2g", bufs=G1B) as gpool, \
                 tc.tile_pool(name="p2ps", bufs=2, space="PSUM") as pps:
                # one-time memset of the gather slots: trailing (pad) slots
                # keep stale SBUF, and garbage bits could be NaN/Inf and
                # poison 0*NaN in the aggregation matmuls.
                for _ in range(G1B):
                    g1t = gpool.tile([P, sched.TWmax, cfg.row_bf], dt.bfloat16,
                                     tag="g1t")
                    nc.vector.memset(g1t.bitcast(dt.float32)[:], 0.0)
                for w in range(cfg.NW):
                    Tw = sched.TW[w]
                    nreal = sched.nreal[w]
                    base = sched.bases[w]
                    rows = min(GR, cfg.NPAD - base)
                    g1t = gpool.tile([P, sched.TWmax, cfg.row_bf], dt.bfloat16,
                                     tag="g1t")
                    # one call per window; single_packet=False because the
                    # ~1100 indices exceed the 64-descriptor packet limit
                    nc.gpsimd.dma_gather(
                        g1t[:, 0:_ceil(nreal, P), :],
                        table[base:base + rows, :],
                        g1i_sb[:, sched.idxcol0[w]:
                               sched.idxcol0[w] + _ceil(nreal, 16)],
                        nreal, nreal, cfg.row_bf, single_packet=False)
                    jb = sched.tile_base[w]
                    oht = pool.tile([P, sched.TWmax, 2, P], oh_mdt, tag="oht")
                    nc.sync.dma_start(out=oht[:, :Tw, :, :],
                                      in_=ohd[:, jb:jb + Tw, :, :])

                    # s_tgt expansion (per tile) via transposed one-hot matmul
                    stgt = pps.tile([P, sched.TWmax, 2, H], dt.float32, tag="stgt")
                    for t in range(Tw):
                        nc.tensor.matmul(stgt[:, t, :, :], oht[:, t, 1, :],
                                         s_hilo[:, w, :, :], start=True, stop=True)
                    g1t32 = g1t.bitcast(dt.float32)
                    s_sum = pool.tile([P, sched.TWmax, H], dt.float32, tag="s_sum")
                    s_act = pool.tile([P, sched.TWmax, H], dt.float32, tag="s_act")
                    nc.vector.tensor_tensor(
                        out=s_sum[:, :Tw, :], in0=stgt[:, :Tw, 0, :],
                        in1=g1t32[:, :Tw, cfg.s_src_off:cfg.s_src_off + H],
                        op=mybir.AluOpType.add)
                    nc.vector.tensor_tensor(
                        out=s_act[:, :Tw, :], in0=stgt[:, :Tw, 1, :],
                        in1=s_sum[:, :Tw, :], op=mybir.AluOpType.add)
                    nc.vector.scalar_tensor_tensor(
                        out=s_sum[:, :Tw, :], in0=s_act[:, :Tw, :], scalar=cfg.leak,
                        in1=s_act[:, :Tw, :], op0=mybir.AluOpType.mult,
                        op1=mybir.AluOpType.max)
                    # exp, expanded to F copies per head on ScalarE (0-stride
                    # read) so the big multiply below runs in DVE 2x mode
                    exp_r = pool.tile([P, sched.TWmax, HID], dt.bfloat16, tag="exp_r")
                    exp_in = _bcast_last(s_sum[:, :Tw, :], cfg.F)
                    nc.scalar.activation(out=exp_r[:, :Tw, :].rearrange(
                                             "p t (h f) -> p t h f", h=H),
                                         in_=exp_in,
                                         func=mybir.ActivationFunctionType.Exp)

                    w_t = pool.tile([P, sched.TWmax, HID], dt.bfloat16, tag="w_t")
                    nc.vector.tensor_tensor(out=w_t[:, :Tw, :],
                                            in0=g1t[:, :Tw, 0:HID],
                                            in1=exp_r[:, :Tw, :],
                                            op=mybir.AluOpType.mult)

                    exp_h = exp_r[:].rearrange("p t (h f) -> p t h f", h=H)
                    agg = pps.tile([P, HID], dt.float32, tag="agg")
                    den = pps.tile([P, H], dt.float32, tag="den")
                    for t in range(Tw):
                        nc.tensor.matmul(agg[:], oht[:, t, 0, :], w_t[:, t, :],
                                         start=(t == 0), stop=(t == Tw - 1))
                        nc.tensor.matmul(den[:], oht[:, t, 0, :], exp_h[:, t, :, 0],
                                         start=(t == 0), stop=(t == Tw - 1))

                    # flush: softmax divide + PReLU
                    den_sb = pool.tile([P, H, 1], dt.float32, tag="den_sb")
                    recip = pool.tile([P, H, 1], dt.float32, tag="recip")
                    nc.vector.tensor_scalar_add(out=den_sb[:, :, 0], in0=den[:],
                                                scalar1=1e-16)
                    nc.vector.reciprocal(out=recip[:], in_=den_sb[:])
                    z = pool.tile([P, HID], dt.float32, tag="z")
                    agg4 = agg[:].rearrange("p (h f) -> p h f", h=H)
                    z4 = z[:].rearrange("p (h f) -> p h f", h=H)
                    nc.vector.tensor_tensor(out=z4, in0=agg4,
                                            in1=_bcast_last(recip[:, :, 0], cfg.F),
                                            op=mybir.AluOpType.mult)
                    res = pool.tile([P, HID], out_mdt, tag="res")
                    nc.vector.scalar_tensor_tensor(
                        out=res[:], in0=z[:], scalar=a_sb[:, 0:1], in1=z[:],
                        op0=mybir.AluOpType.mult, op1=mybir.AluOpType.max)
                    nc.sync.dma_start(out=out[w * P:(w + 1) * P, :], in_=res[:])

    nc.compile()
    return nc


def prepare(cfg: Cfg, inputs):
    """Host-side prep. Returns (sched, in_maps, assemble)."""
    X = np.asarray(inputs["in_nodes_features"], np.float32)
    ei = np.asarray(inputs["edge_index"], np.int64)
    W = np.asarray(inputs["W"], np.float32)
    b_lin = np.asarray(inputs["b_lin"], np.float32)
    a_src = np.asarray(inputs["a_src"], np.float32)
    a_tgt = np.asarray(inputs["a_tgt"], np.float32)
    bias = np.asarray(inputs["bias"], np.float32)
    prelu_a = float(np.asarray(inputs["prelu_a"], np.float32))

    assert np.all(b_lin == 0) and np.all(bias == 0), "nonzero bias unsupported"
    assert 0.0 <= prelu_a <= 1.0, "prelu_a outside [0,1] unsupported"

    src, trg = ei[0], ei[1]
    core_of = trg // cfg.shard

    # per-core edge lists + per-node first/last windows
    core_esrc, core_trel, core_uniq, core_fw, core_lw = [], [], [], [], []
    for k in range(cfg.ncores):
        m = core_of == k
        esrc_k = src[m]
        trel_k = trg[m] - k * cfg.shard
        win_k = trel_k // P
        first = np.full(cfg.N, cfg.NW, np.int64)
        last = np.full(cfg.N, -1, np.int64)
        np.minimum.at(first, esrc_k, win_k)
        np.maximum.at(last, esrc_k, win_k)
        uniq = np.nonzero(last >= 0)[0]
        core_esrc.append(esrc_k)
        core_trel.append(trel_k)
        core_uniq.append(uniq)
        core_fw.append(first[uniq])
        core_lw.append(last[uniq])

    ntc_max = max(len(u) for u in core_uniq)
    cfg.set_table_rows(ntc_max)
    assert cfg.NPAD - GR < GR, "base span must fit the int16 gather range"

    # compile-time window bases: Hall-feasible midpoints, monotone
    lo = np.zeros(cfg.NW, np.int64)
    hi = np.full(cfg.NW, max(0, cfg.NPAD - GR), np.int64)
    for k in range(cfg.ncores):
        U = np.cumsum(np.bincount(core_fw[k], minlength=cfg.NW))   # first<=w
        CL = np.concatenate([[0], np.cumsum(
            np.bincount(core_lw[k], minlength=cfg.NW))[:-1]])      # last<w
        lo = np.maximum(lo, U - GR)
        hi = np.minimum(hi, CL)
    lo = np.maximum(lo, 0)
    assert np.all(lo <= hi), "no feasible window bases"
    bases = np.maximum.accumulate((lo + hi) // 2)

    counts = np.zeros((cfg.ncores, cfg.NW), np.int64)
    core_pos = []
    for k in range(cfg.ncores):
        pos = match_positions(core_fw[k], core_lw[k], bases, cfg.NPAD)
        core_pos.append(pos)
        counts[k] = np.bincount(core_trel[k] // P, minlength=cfg.NW)

    sched = build_schedule(cfg, bases, counts)

    wtp, wap = pack_w(cfg, W, a_src, a_tgt)
    av = np.full((P, 1), prelu_a, np.float32)

    in_maps = []
    for k in range(cfg.ncores):
        remap = np.empty(cfg.N, np.int64)
        remap[core_uniq[k]] = core_pos[k]
        epos_k = remap[core_esrc[k]]
        g1i_k, oh_k = prep_core(cfg, sched, epos_k, core_trel[k])
        Xg = np.zeros((cfg.NPAD, cfg.HID), np.float32)
        Xg[core_pos[k]] = X[core_uniq[k]]
        xt_k = pack_xt(cfg, Xg, cfg.NT)
        xs_k = pack_xt(cfg, X[k * cfg.shard:(k + 1) * cfg.shard], cfg.NW)
        in_maps.append({
            "xt": xt_k, "xs": xs_k, "wt": wtp, "wa": wap,
            "g1i": g1i_k, "ohd": oh_k, "avec": av,
        })

    def assemble(core_outs):
        return np.concatenate(
            [np.asarray(o["out"][: cfg.shard], np.float32) for o in core_outs],
            axis=0)

    return sched, in_maps, assemble


_BUILT = {}


def _get_built(cfg: Cfg, sched: Schedule):
    key = (cfg.N, cfg.E, cfg.HID, cfg.HEADS, cfg.ncores, cfg.NT,
           tuple(sched.TW), tuple(sched.bases), sched.idxcols)
    if key not in _BUILT:
        _BUILT[key] = build_nc(cfg, sched)
    return _BUILT[key]


def kernel(**inputs):
    from concourse.bass_utils import run_bass_kernel_spmd

    cfg = Cfg()
    sched, in_maps, assemble = prepare(cfg, inputs)
    nc = _get_built(cfg, sched)
    res = run_bass_kernel_spmd(nc, in_maps, core_ids=list(range(cfg.ncores)))
    return assemble(res.results)


# revision 36
# speedup vs baseline: 8.3029x; 1.0378x over previous
"""GAT layer (multi-head graph attention) on 8 TRN2 NeuronCores.

Strategy (per sharding hint): destination nodes are sharded across the 8
cores.  Each core:
  phase 0: computes s_tgt for its 12.5k shard targets (small GEMM against
           the a_tgt-folded weight) into a dedicated DRAM tensor.
  phase 1: computes the projection table for the ~63k unique edge-source
           nodes it needs (compacted, per-core order), packed as
           [proj bf16 | s_src f32] rows (1056B used of a 1280B pitch).
           The per-core row ORDER is chosen by an interval matching so
           that every destination window's sources fall inside ONE
           sliding 32768-row range -> a single dma_gather per window.
  phase 2: walks its shard's destination windows (128 targets / window).
           One dma_gather pulls the window's source rows (int16 indices
           against the window's compile-time base), scores -> leaky-relu
           -> exp (ScalarE-expanded to per-feature lanes) run batched
           per window, and fp8 one-hot matmuls (host-streamed)
           accumulate the softmax denominator and the weighted
           aggregation in PSUM.  Softmax division + PReLU at flush.

kernel(**inputs) takes the FULL inputs and returns the FULL output.
"""

import heapq
from dataclasses import dataclass

import numpy as np
import ml_dtypes

BF16 = ml_dtypes.bfloat16
P = 128
GR = 32768        # gather index range (int16)
XJ = 4            # projection tiles per xt DMA block


def _ceil(a, b):
    return -(-a // b)


@dataclass
class Cfg:
    N: int = 100000
    E: int = 800000
    HID: int = 512
    HEADS: int = 8
    ncores: int = 8
    leak: float = 0.01
    oh_bf16: bool = False  # one-hot stream dtype (fp8 halves DMA; bf16 fallback)
    out_bf16: bool = True  # device writes bf16 output (cast to f32 on host)

    def __post_init__(self):
        assert self.N % self.ncores == 0
        self.F = self.HID // self.HEADS
        self.shard = self.N // self.ncores
        self.NW = _ceil(self.shard, P)          # windows per core
        self.NT = _ceil(self.N, P)              # projection tiles (pre-compact)
        self.NPAD = self.NT * P
        self.KP = min(self.HID, P)              # contraction partitions
        self.KT = self.HID // self.KP           # contraction tiles
        row_bytes = self.HID * 2 + 2 * self.HEADS * 4
        self.row_bytes = _ceil(row_bytes, 256) * 256
        self.row_bf = self.row_bytes // 2
        self.row_f32 = self.row_bytes // 4
        self.s_src_off = self.HID // 2          # f32 col of s_src in a row
        self.row_w32 = self.s_src_off + self.HEADS  # f32 cols actually written

    def set_table_rows(self, nrows: int):
        """Size the projection table (compacted per-core source order)."""
        self.NT = _ceil(nrows, P)
        self.NPAD = self.NT * P


@dataclass
class Schedule:
    """Core-independent (uniform) phase-2 schedule: one gather per window."""
    bases: list        # per-window gather base row (compile-time, all cores)
    seg: list          # per-window slot count (128-mult)
    nreal: list        # per-window gather count (16-mult, max over cores)
    idxcol0: list      # per-window idx column offset
    TW: list           # tiles per window
    TWmax: int
    TT: int            # total tiles
    tile_base: list    # first global tile index of each window
    idxcols: int       # total int16 idx columns (16-wrap rows)


def build_schedule(cfg: Cfg, bases, counts) -> Schedule:
    """counts: [ncores, NW] per-window edge counts."""
    maxcnt = counts.max(axis=0)
    nreal, seg, TW, tile_base, idxcol0 = [], [], [], [], []
    idxcol = 0
    tt = 0
    for w in range(cfg.NW):
        nr = max(16, _ceil(int(maxcnt[w]), 16) * 16)
        sg = _ceil(nr, P) * P
        nreal.append(nr)
        seg.append(sg)
        TW.append(sg // P)
        tile_base.append(tt)
        idxcol0.append(idxcol)
        tt += sg // P
        idxcol += sg // 16
    return Schedule(bases=list(bases), seg=seg, nreal=nreal, idxcol0=idxcol0,
                    TW=TW, TWmax=max(TW), TT=tt, tile_base=tile_base,
                    idxcols=idxcol)


def match_positions(nfirst, nlast, bases, npad):
    """Assign each node a table position p with
    bases[last] <= p < bases[first] + GR (interval point matching).

    nfirst/nlast: per-node first/last window. Returns pos array."""
    lo = bases[nlast]                      # lower bound per node
    ub = bases[nfirst] + GR                # exclusive upper bound per node
    order = np.argsort(lo, kind="stable")
    pos = np.empty(len(lo), np.int64)
    heap = []
    oi = 0
    n = len(lo)
    done = 0
    for p in range(npad):
        while oi < n and lo[order[oi]] <= p:
            node = order[oi]
            heapq.heappush(heap, (int(ub[node]), int(node)))
            oi += 1
        if not heap:
            continue
        u, node = heapq.heappop(heap)
        assert u > p, f"interval matching infeasible at position {p}"
        pos[node] = p
        done += 1
        if done == n:
            break
    assert done == n, f"only placed {done}/{n} nodes"
    return pos


def prep_core(cfg: Cfg, sched: Schedule, epos, trel):
    """Per-core input arrays: g1 idx stream and one-hot stream.

    epos: per-edge source POSITION in this core's table.
    trel: per-edge target relative to the core's shard base.
    """
    oh_dt = BF16 if cfg.oh_bf16 else ml_dtypes.float8_e4m3
    win = trel // P
    # order edges by (window, position): position-ascending per window makes
    # the gather's HBM reads address-monotonic (row locality)
    order = np.lexsort((epos, win))
    epos, trel, win = (a[order] for a in (epos, trel, win))

    g1i = np.zeros((P, sched.idxcols), np.int16)
    oh = np.zeros((P, sched.TT, 2, P), oh_dt)

    starts = np.searchsorted(win, np.arange(cfg.NW), side="left")
    ends = np.searchsorted(win, np.arange(cfg.NW), side="right")

    for w in range(cfg.NW):
        lo, hi = int(starts[w]), int(ends[w])
        cnt = hi - lo
        nslots = sched.seg[w]
        assert cnt <= nslots
        rel = epos[lo:hi] - sched.bases[w]
        assert cnt == 0 or (rel.min() >= 0 and rel.max() < GR), \
            f"window {w}: idx out of range"
        idx = np.zeros(nslots, np.int16)
        idx[:cnt] = rel.astype(np.int16)
        if 0 < cnt < nslots:
            idx[cnt:] = idx[cnt - 1]   # pads re-hit the open HBM row
        blk = idx.reshape(nslots // 16, 16).T          # [16, cols]
        c0 = sched.idxcol0[w]
        g1i[:, c0:c0 + nslots // 16] = np.tile(blk, (8, 1))
        # one-hots for this window's tiles
        tloc = (trel[lo:hi] - w * P).astype(np.int64)  # [cnt] in [0,128)
        t0 = sched.tile_base[w]
        for j in range(nslots // P):
            s0, s1 = j * P, min((j + 1) * P, cnt)
            if s1 <= s0:
                continue
            rows = np.arange(s0, s1) - s0
            cols = tloc[s0:s1]
            oh[rows, t0 + j, 0, cols] = oh_dt(1.0)
            oh[cols, t0 + j, 1, rows] = oh_dt(1.0)
    return g1i, oh


def pack_xt(cfg: Cfg, X: np.ndarray, nt: int) -> np.ndarray:
    """X [rows, HID] f32 -> bf16 packed [KP, NJ, KT, XJ*P]:
    (p, jb, ki, j*P+n) = X[(jb*XJ+j)*P+n, ki*KP+p]."""
    nj = _ceil(nt, XJ)
    Xp = np.zeros((nj * XJ * P, cfg.HID), np.float32)
    Xp[: X.shape[0]] = np.asarray(X, np.float32)
    Xb = Xp.astype(BF16)
    v = Xb.reshape(nj, XJ * P, cfg.KT, cfg.KP)
    return np.ascontiguousarray(v.transpose(3, 0, 2, 1))


def pack_w(cfg: Cfg, W, a_src, a_tgt):
    """Returns wt [KP, KT, HID] bf16 and wa [KP, KT, 2*HEADS] bf16."""
    WT = W.T.astype(np.float32)                       # [HID(d), HID(o)]
    wa_s = (W.reshape(cfg.HEADS, cfg.F, cfg.HID)
            * np.asarray(a_src, np.float32).reshape(cfg.HEADS, cfg.F, 1)).sum(1)
    wa_t = (W.reshape(cfg.HEADS, cfg.F, cfg.HID)
            * np.asarray(a_tgt, np.float32).reshape(cfg.HEADS, cfg.F, 1)).sum(1)
    WA = np.concatenate([wa_s.T, wa_t.T], axis=1)     # [d, 2H]
    wt = np.ascontiguousarray(
        WT.astype(BF16).reshape(cfg.KT, cfg.KP, cfg.HID).transpose(1, 0, 2))
    wa = np.ascontiguousarray(
        WA.astype(BF16).reshape(cfg.KT, cfg.KP, 2 * cfg.HEADS).transpose(1, 0, 2))
    return wt, wa


def _bcast_last(ap, n):
    """Append a 0-stride broadcast dim of size n to an AP."""
    import concourse.bass as bass
    lst = [list(x) for x in ap.ap] + [[0, n]]
    return bass.AP(ap.tensor, ap.offset, lst)


def build_nc(cfg: Cfg, sched: Schedule):
    import concourse.bacc as bacc
    import concourse.mybir as mybir
    from concourse.tile import TileContext

    dt = mybir.dt
    oh_mdt = dt.bfloat16 if cfg.oh_bf16 else dt.float8e4
    out_mdt = dt.bfloat16 if cfg.out_bf16 else dt.float32
    H, HID, KT, KP = cfg.HEADS, cfg.HID, cfg.KT, cfg.KP

    nc = bacc.Bacc("TRN2", target_bir_lowering=False)

    NJ = _ceil(cfg.NT, XJ)
    NJ2 = _ceil(cfg.NW, XJ)
    xt = nc.dram_tensor("xt", [KP, NJ, KT, XJ * P], dt.bfloat16,
                        kind="ExternalInput")
    xs = nc.dram_tensor("xs", [KP, NJ2, KT, XJ * P], dt.bfloat16,
                        kind="ExternalInput")
    wt = nc.dram_tensor("wt", [KP, KT, HID], dt.bfloat16, kind="ExternalInput")
    wa = nc.dram_tensor("wa", [KP, KT, 2 * H], dt.bfloat16, kind="ExternalInput")
    g1i = nc.dram_tensor("g1i", [P, sched.idxcols], dt.int16, kind="ExternalInput")
    ohd = nc.dram_tensor("ohd", [P, sched.TT, 2, P], oh_mdt, kind="ExternalInput")
    avec = nc.dram_tensor("avec", [P, 1], dt.float32, kind="ExternalInput")
    out = nc.dram_tensor("out", [cfg.NW * P, HID], out_mdt, kind="ExternalOutput")

    with TileContext(nc) as tc:
        with tc.tile_pool(name="const", bufs=1) as cpool, \
             tc.tile_pool(name="dram", bufs=1, space="DRAM") as dpool:
            table = dpool.tile([cfg.NPAD, cfg.row_bf], dt.bfloat16)
            stgt_d = dpool.tile([NJ2 * XJ * P, H], dt.float32)
            wt_sb = cpool.tile([KP, KT, HID], dt.bfloat16)
            nc.sync.dma_start(out=wt_sb[:], in_=wt[:, :, :])
            wa_sb = cpool.tile([KP, KT, 2 * H], dt.bfloat16)
            nc.sync.dma_start(out=wa_sb[:], in_=wa[:, :, :])
            a_sb = cpool.tile([P, 1], dt.float32)
            nc.sync.dma_start(out=a_sb[:], in_=avec[:, :])
            g1i_sb = cpool.tile([P, sched.idxcols], dt.int16)
            nc.sync.dma_start(out=g1i_sb[:], in_=g1i[:, :])

            # ---------- phase 0: s_tgt for the local shard ----------
            with tc.tile_pool(name="p0", bufs=2) as x2pool, \
                 tc.tile_pool(name="p0ps", bufs=2, space="PSUM") as psp0, \
                 tc.tile_pool(name="p0st", bufs=2) as st0pool:
                for jb in range(NJ2):
                    xtile = x2pool.tile([KP, KT, XJ * P], dt.bfloat16, tag="x2")
                    nc.sync.dma_start(out=xtile[:], in_=xs[:, jb, :, :])
                    stg3 = st0pool.tile([P, XJ, H], dt.float32, tag="stg3")
                    for j in range(XJ):
                        ps3 = psp0.tile([P, H], dt.float32, tag="ps3")
                        for ki in range(KT):
                            nc.tensor.matmul(ps3[:],
                                             xtile[:, ki, j * P:(j + 1) * P],
                                             wa_sb[:, ki, H:2 * H],
                                             start=(ki == 0), stop=(ki == KT - 1))
                        nc.scalar.copy(out=stg3[:, j, :], in_=ps3[:])
                    dst = stgt_d[jb * XJ * P:(jb + 1) * XJ * P, :].rearrange(
                        "(j p) h -> p j h", p=P)
                    nc.sync.dma_start(out=dst, in_=stg3[:])

            # ---------- phase 1: projection table (sources) ----------
            # rows are written 1056B-dense (proj + s_src); the DRAM row
            # tail (1056:1280) stays uninitialized and is never read.
            with tc.tile_pool(name="p1", bufs=2) as xpool, \
                 tc.tile_pool(name="p1ps", bufs=3, space="PSUM") as psp1, \
                 tc.tile_pool(name="p1ps2", bufs=2, space="PSUM") as psp2, \
                 tc.tile_pool(name="p1st", bufs=3) as stpool:
                SJ = 2  # staging tiles per table write
                for jb in range(NJ):
                    xtile = xpool.tile([KP, KT, XJ * P], dt.bfloat16, tag="x")
                    nc.sync.dma_start(out=xtile[:], in_=xt[:, jb, :, :])
                    for j in range(XJ):
                        ps1 = psp1.tile([P, HID], dt.float32, tag="ps1")
                        ps2 = psp2.tile([P, H], dt.float32, tag="ps2")
                        for ki in range(KT):
                            nc.tensor.matmul(ps1[:], xtile[:, ki, j * P:(j + 1) * P],
                                             wt_sb[:, ki, :],
                                             start=(ki == 0), stop=(ki == KT - 1))
                            nc.tensor.matmul(ps2[:], xtile[:, ki, j * P:(j + 1) * P],
                                             wa_sb[:, ki, 0:H],
                                             start=(ki == 0), stop=(ki == KT - 1))
                        jj = jb * XJ + j
                        if jj % SJ == 0:
                            stg = stpool.tile([P, SJ, cfg.row_bf], dt.bfloat16,
                                              tag="stg")
                            stg32 = stg.bitcast(dt.float32)
                        sl = jj % SJ
                        nc.scalar.copy(out=stg[:, sl, 0:HID], in_=ps1[:])
                        nc.scalar.copy(
                            out=stg32[:, sl, cfg.s_src_off:cfg.s_src_off + H],
                            in_=ps2[:])
                        if sl == SJ - 1:
                            r0 = (jj - sl) * P
                            dst = table[r0:r0 + SJ * P, 0:2 * cfg.row_w32].rearrange(
                                "(j p) c -> p j c", p=P)
                            nc.sync.dma_start(out=dst,
                                              in_=stg[:, :, 0:2 * cfg.row_w32])

            # no inter-phase barrier: Tile's dependency tracking orders the
            # gathers behind the table writes; if it tracks regions, early
            # windows' gathers overlap the tail of phase 1

            # ---------- phase 1.5: resident s_tgt (hi/lo bf16) ----------
            s_ap = stgt_d[0:cfg.NW * P, :].rearrange("(w p) h -> p w h", p=P)
            s_all = cpool.tile([P, cfg.NW, H], dt.float32)
            nc.sync.dma_start(out=s_all[:], in_=s_ap)
            s_hilo = cpool.tile([P, cfg.NW, 2, H], dt.bfloat16)
            s_hi32 = cpool.tile([P, cfg.NW, H], dt.float32)
            nc.vector.tensor_copy(out=s_hilo[:, :, 0, :], in_=s_all[:])
            nc.vector.tensor_copy(out=s_hi32[:], in_=s_hilo[:, :, 0, :])
            nc.vector.tensor_tensor(out=s_hilo[:, :, 1, :], in0=s_all[:],
                                    in1=s_hi32[:], op=mybir.AluOpType.subtract)

            # ---------- phase 2: windows ----------
            G1B = 6
            with tc.tile_pool(name="p2", bufs=2) as pool, \
                 tc.tile_pool(name="p2g", bufs=G1B) as gpool, \
                 tc.tile_pool(name="p2ps", bufs=2, space="PSUM") as pps:
                # one-time memset of the gather slots: trailing (pad) slots
                # keep stale SBUF, and garbage bits could be NaN/Inf and
                # poison 0*NaN in the aggregation matmuls.
                for _ in range(G1B):
                    g1t = gpool.tile([P, sched.TWmax, cfg.row_bf], dt.bfloat16,
                                     tag="g1t")
                    nc.vector.memset(g1t.bitcast(dt.float32)[:], 0.0)
                for w in range(cfg.NW):
                    Tw = sched.TW[w]
                    nreal = sched.nreal[w]
                    base = sched.bases[w]
                    rows = min(GR, cfg.NPAD - base)
                    g1t = gpool.tile([P, sched.TWmax, cfg.row_bf], dt.bfloat16,
                                     tag="g1t")
                    # one call per window; single_packet=False because the
                    # ~1100 indices exceed the 64-descriptor packet limit
                    nc.gpsimd.dma_gather(
                        g1t[:, 0:_ceil(nreal, P), :],
                        table[base:base + rows, :],
                        g1i_sb[:, sched.idxcol0[w]:
                               sched.idxcol0[w] + _ceil(nreal, 16)],
                        nreal, nreal, cfg.row_bf, single_packet=False)
                    jb = sched.tile_base[w]
                    oht = pool.tile([P, sched.TWmax, 2, P], oh_mdt, tag="oht")
                    nc.sync.dma_start(out=oht[:, :Tw, :, :],
                                      in_=ohd[:, jb:jb + Tw, :, :])

                    # s_tgt expansion (per tile) via transposed one-hot matmul
                    stgt = pps.tile([P, sched.TWmax, 2, H], dt.float32, tag="stgt")
                    for t in range(Tw):
                        nc.tensor.matmul(stgt[:, t, :, :], oht[:, t, 1, :],
                                         s_hilo[:, w, :, :], start=True, stop=True)
                    g1t32 = g1t.bitcast(dt.float32)
                    s_sum = pool.tile([P, sched.TWmax, H], dt.float32, tag="s_sum")
                    s_act = pool.tile([P, sched.TWmax, H], dt.float32, tag="s_act")
                    nc.vector.tensor_tensor(
                        out=s_sum[:, :Tw, :], in0=stgt[:, :Tw, 0, :],
                        in1=g1t32[:, :Tw, cfg.s_src_off:cfg.s_src_off + H],
                        op=mybir.AluOpType.add)
                    nc.vector.tensor_tensor(
                        out=s_act[:, :Tw, :], in0=stgt[:, :Tw, 1, :],
                        in1=s_sum[:, :Tw, :], op=mybir.AluOpType.add)
                    nc.vector.scalar_tensor_tensor(
                        out=s_sum[:, :Tw, :], in0=s_act[:, :Tw, :], scalar=cfg.leak,
                        in1=s_act[:, :Tw, :], op0=mybir.AluOpType.mult,
                        op1=mybir.AluOpType.max)
                    # exp, expanded to F copies per head on ScalarE (0-stride
                    # read) so the big multiply below runs in DVE 2x mode
                    exp_r = pool.tile([P, sched.TWmax, HID], dt.bfloat16, tag="exp_r")
                    exp_in = _bcast_last(s_sum[:, :Tw, :], cfg.F)
                    nc.scalar.activation(out=exp_r[:, :Tw, :].rearrange(
                                             "p t (h f) -> p t h f", h=H),
                                         in_=exp_in,
                                         func=mybir.ActivationFunctionType.Exp)

                    w_t = pool.tile([P, sched.TWmax, HID], dt.bfloat16, tag="w_t")
                    nc.vector.tensor_tensor(out=w_t[:, :Tw, :],
                                            in0=g1t[:, :Tw, 0:HID],
                                            in1=exp_r[:, :Tw, :],
                                            op=mybir.AluOpType.mult)

                    exp_h = exp_r[:].rearrange("p t (h f) -> p t h f", h=H)
                    agg = pps.tile([P, HID], dt.float32, tag="agg")
                    den = pps.tile([P, H], dt.float32, tag="den")
                    for t in range(Tw):
                        nc.tensor.matmul(agg[:], oht[:, t, 0, :], w_t[:, t, :],
                                         start=(t == 0), stop=(t == Tw - 1))
                        nc.tensor.matmul(den[:], oht[:, t, 0, :], exp_h[:, t, :, 0],
                                         start=(t == 0), stop=(t == Tw - 1))

                    # flush: softmax divide + PReLU
                    den_sb = pool.tile([P, H, 1], dt.float32, tag="den_sb")
                    recip = pool.tile([P, H, 1], dt.float32, tag="recip")
                    nc.vector.tensor_scalar_add(out=den_sb[:, :, 0], in0=den[:],
                                                scalar1=1e-16)
                    nc.vector.reciprocal(out=recip[:], in_=den_sb[:])
                    z = pool.tile([P, HID], dt.float32, tag="z")
                    agg4 = agg[:].rearrange("p (h f) -> p h f", h=H)
                    z4 = z[:].rearrange("p (h f) -> p h f", h=H)
                    nc.vector.tensor_tensor(out=z4, in0=agg4,
                                            in1=_bcast_last(recip[:, :, 0], cfg.F),
                                            op=mybir.AluOpType.mult)
                    res = pool.tile([P, HID], out_mdt, tag="res")
                    nc.vector.scalar_tensor_tensor(
                        out=res[:], in0=z[:], scalar=a_sb[:, 0:1], in1=z[:],
                        op0=mybir.AluOpType.mult, op1=mybir.AluOpType.max)
                    nc.sync.dma_start(out=out[w * P:(w + 1) * P, :], in_=res[:])

    nc.compile()
    return nc


def prepare(cfg: Cfg, inputs):
    """Host-side prep. Returns (sched, in_maps, assemble)."""
    X = np.asarray(inputs["in_nodes_features"], np.float32)
    ei = np.asarray(inputs["edge_index"], np.int64)
    W = np.asarray(inputs["W"], np.float32)
    b_lin = np.asarray(inputs["b_lin"], np.float32)
    a_src = np.asarray(inputs["a_src"], np.float32)
    a_tgt = np.asarray(inputs["a_tgt"], np.float32)
    bias = np.asarray(inputs["bias"], np.float32)
    prelu_a = float(np.asarray(inputs["prelu_a"], np.float32))

    assert np.all(b_lin == 0) and np.all(bias == 0), "nonzero bias unsupported"
    assert 0.0 <= prelu_a <= 1.0, "prelu_a outside [0,1] unsupported"

    src, trg = ei[0], ei[1]
    core_of = trg // cfg.shard

    # per-core edge lists + per-node first/last windows
    core_esrc, core_trel, core_uniq, core_fw, core_lw = [], [], [], [], []
    for k in range(cfg.ncores):
        m = core_of == k
        esrc_k = src[m]
        trel_k = trg[m] - k * cfg.shard
        win_k = trel_k // P
        first = np.full(cfg.N, cfg.NW, np.int64)
        last = np.full(cfg.N, -1, np.int64)
        np.minimum.at(first, esrc_k, win_k)
        np.maximum.at(last, esrc_k, win_k)
        uniq = np.nonzero(last >= 0)[0]
        core_esrc.append(esrc_k)
        core_trel.append(trel_k)
        core_uniq.append(uniq)
        core_fw.append(first[uniq])
        core_lw.append(last[uniq])

    ntc_max = max(len(u) for u in core_uniq)
    cfg.set_table_rows(ntc_max)
    assert cfg.NPAD - GR < GR, "base span must fit the int16 gather range"

    # compile-time window bases: Hall-feasible midpoints, monotone
    lo = np.zeros(cfg.NW, np.int64)
    hi = np.full(cfg.NW, max(0, cfg.NPAD - GR), np.int64)
    for k in range(cfg.ncores):
        U = np.cumsum(np.bincount(core_fw[k], minlength=cfg.NW))   # first<=w
        CL = np.concatenate([[0], np.cumsum(
            np.bincount(core_lw[k], minlength=cfg.NW))[:-1]])      # last<w
        lo = np.maximum(lo, U - GR)
        hi = np.minimum(hi, CL)
    lo = np.maximum(lo, 0)
    assert np.all(lo <= hi), "no feasible window bases"
    bases = np.maximum.accumulate((lo + hi) // 2)

    counts = np.zeros((cfg.ncores, cfg.NW), np.int64)
    core_pos = []
    for k in range(cfg.ncores):
        pos = match_positions(core_fw[k], core_lw[k], bases, cfg.NPAD)
        core_pos.append(pos)
        counts[k] = np.bincount(core_trel[k] // P, minlength=cfg.NW)

    sched = build_schedule(cfg, bases, counts)

    wtp, wap = pack_w(cfg, W, a_src, a_tgt)
    av = np.full((P, 1), prelu_a, np.float32)

    in_maps = []
    for k in range(cfg.ncores):
        remap = np.empty(cfg.N, np.int64)
        remap[core_uniq[k]] = core_pos[k]
        epos_k = remap[core_esrc[k]]
        g1i_k, oh_k = prep_core(cfg, sched, epos_k, core_trel[k])
        Xg = np.zeros((cfg.NPAD, cfg.HID), np.float32)
        Xg[core_pos[k]] = X[core_uniq[k]]
        xt_k = pack_xt(cfg, Xg, cfg.NT)
        xs_k = pack_xt(cfg, X[k * cfg.shard:(k + 1) * cfg.shard], cfg.NW)
        in_maps.append({
            "xt": xt_k, "xs": xs_k, "wt": wtp, "wa": wap,
            "g1i": g1i_k, "ohd": oh_k, "avec": av,
        })

    def assemble(core_outs):
        return np.concatenate(
            [np.asarray(o["out"][: cfg.shard], np.float32) for o in core_outs],
            axis=0)

    return sched, in_maps, assemble


_BUILT = {}


def _get_built(cfg: Cfg, sched: Schedule):
    key = (cfg.N, cfg.E, cfg.HID, cfg.HEADS, cfg.ncores, cfg.NT,
           tuple(sched.TW), tuple(sched.bases), sched.idxcols)
    if key not in _BUILT:
        _BUILT[key] = build_nc(cfg, sched)
    return _BUILT[key]


def kernel(**inputs):
    from concourse.bass_utils import run_bass_kernel_spmd

    cfg = Cfg()
    sched, in_maps, assemble = prepare(cfg, inputs)
    nc = _get_built(cfg, sched)
    res = run_bass_kernel_spmd(nc, in_maps, core_ids=list(range(cfg.ncores)))
    return assemble(res.results)
